# revision 10
# baseline (speedup 1.0000x reference)
"""Trainium2 Bass kernel for CustomRoPEAttention (B=2, S=2048, H=16, Dh=128).

Sharding: 8 cores = 2 batches x 4 head-groups (4 heads/core). Each core:
QKV projection (bf16 matmuls) + RoPE for its heads, transposed-layout causal
attention (scores computed as S^T with keys on partitions so the attention
probabilities feed A@V directly as the moving operand -- no PE transposes),
softmax denominators via ap-size-1 matmuls + deferred normalization, and a
partial (transposed) output projection. Host sums the 4 partials per batch.

Self-contained: hardcodes shapes from the problem spec.
"""
import math
from contextlib import ExitStack

import numpy as np
import ml_dtypes

import concourse.mybir as mybir
import concourse.tile as tile
from concourse import bacc
from concourse.bass_utils import run_bass_kernel_spmd
from concourse.masks import make_identity

S = 2048            # sequence
D = 2048            # hidden
NH = 16             # total heads
DH = 128            # head dim
HG = 4              # heads per core
GQ = HG * DH        # 512: per-core q/k/v feature width
B = 2
NCORES = 8
ROPE_THETA = 10000.0
SCALE = 1.0 / math.sqrt(DH)
NEG = -1.0e9
SLAB = 512          # phase-1 sequence slab width
F32 = mybir.dt.float32
BF16 = mybir.dt.bfloat16
F16 = mybir.dt.float16
MULT = mybir.AluOpType.mult
ADD = mybir.AluOpType.add
NB = S // 128       # 16 k/q blocks


def build_nc(reps=1, knobs=None):
    kn = {"p1ps": 3, "p1vps": 2, "p1x": 2, "sps": 2, "avps": 2, "p3ps": 4}
    if knobs:
        kn.update(knobs)
    nc = bacc.Bacc(None, target_bir_lowering=False)
    xt = nc.dram_tensor("xt", [16, 128, S], BF16, kind="ExternalInput")
    wqk = nc.dram_tensor("wqk", [16, 128, 2 * GQ], BF16, kind="ExternalInput")
    wv = nc.dram_tensor("wv", [16, 128, GQ], BF16, kind="ExternalInput")
    wo = nc.dram_tensor("wo", [4, 128, D], BF16, kind="ExternalInput")
    bqkt = nc.dram_tensor("bqkt", [128, 8], F32, kind="ExternalInput")
    bqkt_sw = nc.dram_tensor("bqkt_sw", [128, 8], F32, kind="ExternalInput")
    bv = nc.dram_tensor("bv", [1, GQ], F32, kind="ExternalInput")
    cost = nc.dram_tensor("cost", [128, S], BF16, kind="ExternalInput")    # cos^T
    sinrt = nc.dram_tensor("sinrt", [128, S], BF16, kind="ExternalInput")  # sin^T, rot sign
    maskd = nc.dram_tensor("maskd", [128, 128], F32, kind="ExternalInput")  # tril(-1) NEG
    onesb = nc.dram_tensor("onesb", [128, 1], BF16, kind="ExternalInput")
    outt = nc.dram_tensor("outt", [16, 128, S], BF16, kind="ExternalOutput")
    lrt = nc.dram_tensor("lrt", [HG, 16, 128], F32)  # recip bounce: [16,128] -> [1,2048]

    with tile.TileContext(nc) as tc, ExitStack() as top:
        g = top.enter_context(tc.tile_pool(name="glob", bufs=1))
        tcos = g.tile([128, S], BF16)
        nc.sync.dma_start(out=tcos, in_=cost[:])
        tsin = g.tile([128, S], BF16)
        nc.sync.dma_start(out=tsin, in_=sinrt[:])
        tmask = g.tile([128, 128], F32)
        nc.sync.dma_start(out=tmask, in_=maskd[:])
        ident_f = g.tile([128, 128], F32)
        make_identity(nc, ident_f[:])
        tbqkt = g.tile([128, 8], F32)
        nc.sync.dma_start(out=tbqkt, in_=bqkt[:])
        tbqkt_sw = g.tile([128, 8], F32)
        nc.sync.dma_start(out=tbqkt_sw, in_=bqkt_sw[:])
        tbvb = g.tile([128, GQ], F32)
        nc.sync.dma_start(out=tbvb, in_=bv[:].to_broadcast((128, GQ)))
        tones = g.tile([128, 1], BF16)
        nc.sync.dma_start(out=tones, in_=onesb[:])

        # Whole-kernel residents
        res = top.enter_context(tc.tile_pool(name="res", bufs=1))
        qt = []  # mt 0..3 = Q^T heads, 4..7 = K^T heads, each [128(dh), S] bf16
        for mt in range(2 * HG):
            qt.append(res.tile([128, S], BF16, tag=f"qt{mt}", name=f"qt{mt}"))
        vres = []  # 16 V k-block tiles [128(seq), GQ] bf16
        for t in range(NB):
            vres.append(res.tile([128, GQ], BF16, tag=f"v{t}", name=f"v{t}"))
        two = res.tile([128, 4, D], BF16, tag="two")
        nc.sync.dma_start(out=two, in_=wo.rearrange("kc p f -> p kc f"))
        ct_sb = {}
        for h in range(HG):
            for gq in range(4):
                ct_sb[(h, gq)] = res.tile([128, 512], BF16, tag=f"ct_{h}_{gq}", name=f"ct_{h}_{gq}")

        for _rep in range(reps):
          # ---------------- Phase 1: QKV^T projection + RoPE (all resident) ----------
          with tc.tile_pool(name="p1w", bufs=1) as p1w, \
               tc.tile_pool(name="p1x", bufs=kn["p1x"]) as p1x, \
               tc.tile_pool(name="qswp", bufs=2) as qswp, \
               tc.tile_pool(name="p1stg", bufs=1) as p1stg, \
               tc.tile_pool(name="p1ps", bufs=kn["p1ps"], space="PSUM") as p1ps, \
               tc.tile_pool(name="p1vps", bufs=kn["p1vps"], space="PSUM") as p1vps:
            twqk = []
            twv = []
            for kc in range(16):
                wqkt = p1w.tile([128, 2 * GQ], BF16, tag=f"wqk{kc}")
                nc.sync.dma_start(out=wqkt, in_=wqk[kc])
                twqk.append(wqkt)
                wvt = p1w.tile([128, GQ], BF16, tag=f"wv{kc}")
                nc.sync.dma_start(out=wvt, in_=wv[kc])
                twv.append(wvt)
            for ns in range(S // SLAB):
                sl = slice(ns * SLAB, (ns + 1) * SLAB)
                xs = p1x.tile([128, 16, SLAB], BF16, tag="xs")
                nc.sync.dma_start(out=xs, in_=xt[:, :, sl].rearrange("kc p s -> p kc s"))
                for mt in range(2 * HG):
                    pqk = p1ps.tile([128, SLAB], F32, tag="qkps")
                    for kc in range(16):
                        nc.tensor.matmul(pqk[:], twqk[kc][:, mt * 128:(mt + 1) * 128],
                                         xs[:, kc, :], start=(kc == 0), stop=(kc == 15))
                    nc.scalar.copy(out=qt[mt][:, sl], in_=pqk[:])
                for st in range(SLAB // 128):
                    pv = p1vps.tile([128, GQ], F32, tag="vps")
                    s0 = st * 128
                    for kc in range(16):
                        nc.tensor.matmul(pv[:], xs[:, kc, s0:s0 + 128],
                                         twv[kc][:], start=(kc == 0), stop=(kc == 15))
                    nc.vector.tensor_tensor(out=vres[ns * (SLAB // 128) + st],
                                            in0=pv[:], in1=tbvb[:], op=ADD)
            # RoPE per head tile (full width): q' = (q+b)*cos + swap(q+b)*sin_rot
            for mt in range(2 * HG):
                qsw = qswp.tile([128, S], BF16, tag="qsw")
                nc.sync.dma_start(out=qsw[0:64, :], in_=qt[mt][64:128, :])
                nc.sync.dma_start(out=qsw[64:128, :], in_=qt[mt][0:64, :])
                m1 = p1stg.tile([128, S], BF16, tag="m1")
                nc.vector.scalar_tensor_tensor(
                    out=m1[:], in0=qt[mt][:], scalar=tbqkt[:, mt:mt + 1],
                    in1=tcos[:], op0=ADD, op1=MULT)
                m2 = p1stg.tile([128, S], BF16, tag="m2")
                nc.vector.scalar_tensor_tensor(
                    out=m2[:], in0=qsw[:], scalar=tbqkt_sw[:, mt:mt + 1],
                    in1=tsin[:], op0=ADD, op1=MULT)
                nc.vector.tensor_tensor(out=qt[mt][:], in0=m1[:], in1=m2[:], op=ADD)

          # ---------------- Phase 2: transposed attention ----------------
          p2stack = ExitStack()
          expp = p2stack.enter_context(tc.tile_pool(name="expp", bufs=2))
          lrp = p2stack.enter_context(tc.tile_pool(name="lrp", bufs=2))
          avps = p2stack.enter_context(
              tc.tile_pool(name="avps", bufs=kn["avps"], space="PSUM"))
          rbp = p2stack.enter_context(tc.tile_pool(name="rbp", bufs=2))
          p2inner = ExitStack()
          sps = p2inner.enter_context(
              tc.tile_pool(name="sps", bufs=kn["sps"], space="PSUM"))
          smps = p2inner.enter_context(tc.tile_pool(name="smps", bufs=1, space="PSUM"))

          expT = [None] * HG  # per live head: list of 16 exp(S^T) tiles
          recrow = [None] * HG

          def denom(h, b):
              # ell[q] for q-block b: sum_k exp tiles, ap-1 matmuls, then recip
              lp = smps.tile([128, 1], F32, tag="lp")
              for j in range(b + 1):
                  nc.tensor.matmul(lp[:], expT[h][j][:, (b - j) * 128:(b - j + 1) * 128],
                                   tones[:], start=(j == 0), stop=(j == b))
              nc.vector.reciprocal(out=lrec_cur[h][:, b:b + 1], in_=lp[:])

          def sweep2_group(h, gq):
              # normalize+accumulate: ct = (sum_k V^T[k] expS^T[k]) * recip(ell)
              rbs = rbp.tile([128, 512], F32, tag="rbs")
              nc.gpsimd.partition_broadcast(
                  rbs[:], recrow[h][:, gq * 512:(gq + 1) * 512])
              ct = avps.tile([128, 512], F32, tag="ct")
              last = 4 * gq + 3
              for j in range(last + 1):
                  if j <= 4 * gq:
                      nc.tensor.matmul(ct[:], vres[j][:, h * 128:(h + 1) * 128],
                                       expT[h][j][:, (4 * gq - j) * 128:(4 * gq - j) * 128 + 512],
                                       start=(j == 0), stop=(j == last))
                  else:
                      w = (4 * gq + 4 - j) * 128
                      nc.tensor.matmul(ct[:, 512 - w:512], vres[j][:, h * 128:(h + 1) * 128],
                                       expT[h][j][:, 0:w], start=False, stop=(j == last))
              nc.vector.tensor_tensor(out=ct_sb[(h, gq)][:], in0=ct[:], in1=rbs[:], op=MULT)

          lrec_cur = {}
          for h in range(HG):
              expT[h] = []
              lrec_cur[h] = lrp.tile([128, 16], F32, tag="lrec", name="lrec")
              recrow[h] = lrp.tile([1, S], F32, tag="recrow", name="recrow")
              for i in range(NB):
                  w = (NB - i) * 128
                  ex = expp.tile([128, w], BF16, tag=f"expT{i}", name=f"expT{i}")
                  expT[h].append(ex)
                  for c0 in range(0, w, 1024):
                      cw = min(1024, w - c0)
                      sp = sps.tile([128, 1024], F32, tag="sp")
                      for s5 in range(0, cw, 512):
                          w5 = min(512, cw - s5)
                          q0 = i * 128 + c0 + s5
                          nc.tensor.matmul(sp[:, s5:s5 + w5],
                                           qt[HG + h][:, i * 128:(i + 1) * 128],
                                           qt[h][:, q0:q0 + w5], start=True, stop=True)
                      if c0 == 0:
                          nc.vector.tensor_tensor(out=sp[:, 0:128], in0=sp[:, 0:128],
                                                  in1=tmask[:], op=ADD)
                      nc.scalar.activation(out=ex[:, c0:c0 + cw], in_=sp[:, 0:cw],
                                           func=mybir.ActivationFunctionType.Exp,
                                           scale=SCALE)
                  if i >= 2:
                      denom(h, i - 2)
                  if h >= 1 and i % 4 == 3:
                      sweep2_group(h - 1, i // 4)
              denom(h, NB - 2)
              denom(h, NB - 1)
              # recip row: [128,16] -> transpose -> [16,128] -> DRAM -> [1,2048]
              rt = smps.tile([16, 128], F32, tag="rt")
              nc.tensor.transpose(rt[:], lrec_cur[h][:], ident_f[:])
              rts = lrp.tile([16, 128], F32, tag="rts")
              nc.vector.tensor_copy(out=rts[:], in_=rt[:])
              nc.sync.dma_start(out=lrt[h], in_=rts[:])
              nc.sync.dma_start(out=recrow[h][:].rearrange("one s -> (one s)"),
                                in_=lrt[h].rearrange("a b -> (a b)"))

          # close S^T/denom psum pools before opening phase-3 psum
          p2inner.close()

          # ---------------- Phase 3: output projection, interleaving head-3 sweep2 ----
          with tc.tile_pool(name="p3s", bufs=4) as p3s, \
               tc.tile_pool(name="p3ps", bufs=kn["p3ps"], space="PSUM") as p3ps:
              for gq in range(4):
                  sweep2_group(HG - 1, gq)
                  for mt in range(16):
                      op = p3ps.tile([128, 512], F32, tag="op")
                      for kh in range(HG):
                          nc.tensor.matmul(op[:], two[:, kh, mt * 128:(mt + 1) * 128],
                                           ct_sb[(kh, gq)][:],
                                           start=(kh == 0), stop=(kh == 3))
                      ob = p3s.tile([128, 512], BF16, tag="ob")
                      if mt % 2 == 0:
                          nc.vector.tensor_copy(out=ob[:], in_=op[:])
                      else:
                          nc.scalar.copy(out=ob[:], in_=op[:])
                      nc.sync.dma_start(out=outt[mt, :, gq * 512:(gq + 1) * 512], in_=ob[:])
          p2stack.close()
    nc.finalize()
    return nc


_NC_CACHE = {}


def _get_nc(reps=1):
    if reps not in _NC_CACHE:
        _NC_CACHE[reps] = build_nc(reps)
    return _NC_CACHE[reps]


def _rope_tables(position_ids_b):
    pos = position_ids_b.astype(np.float32)
    inv_freq = (1.0 / (ROPE_THETA ** (np.arange(0, DH, 2, dtype=np.float32) / np.float32(DH))))
    ang = pos[:, None] * inv_freq[None, :]          # [S, 64]
    emb = np.concatenate([ang, ang], axis=-1)       # [S, 128]
    cosT = np.ascontiguousarray(np.cos(emb).T)      # [128, S]
    sinT = np.sin(emb).T
    sin_rot = np.concatenate([-sinT[0:64], sinT[64:128]], axis=0)
    return cosT.astype(ml_dtypes.bfloat16), np.ascontiguousarray(sin_rot).astype(ml_dtypes.bfloat16)


def _make_in_maps(inputs):
    hidden_states = np.asarray(inputs["hidden_states"], dtype=np.float32)
    position_ids = np.asarray(inputs["position_ids"])
    Wqkv = np.asarray(inputs["Wqkv"], dtype=np.float32)
    bqkv = np.asarray(inputs["bqkv"], dtype=np.float32)
    Wo = np.asarray(inputs["Wo"], dtype=np.float32)

    mask = np.tril(np.full((128, 128), NEG, dtype=np.float32), k=-1)
    tabs = [_rope_tables(np.asarray(position_ids)[b]) for b in range(B)]
    xts = [np.ascontiguousarray(hidden_states[b].T).astype(ml_dtypes.bfloat16)
           .reshape(16, 128, S) for b in range(B)]
    onesb = np.ones((128, 1), dtype=ml_dtypes.bfloat16)

    in_maps = []
    for c in range(NCORES):
        b, hg = divmod(c, HG)
        qcols = slice(hg * GQ, (hg + 1) * GQ)
        kcols = slice(D + hg * GQ, D + (hg + 1) * GQ)
        vcols = slice(2 * D + hg * GQ, 2 * D + (hg + 1) * GQ)
        wqk_c = np.ascontiguousarray(
            np.concatenate([Wqkv[:, qcols], Wqkv[:, kcols]], axis=1)
        ).astype(ml_dtypes.bfloat16).reshape(16, 128, 2 * GQ)
        wv_c = np.ascontiguousarray(Wqkv[:, vcols]).astype(ml_dtypes.bfloat16).reshape(16, 128, GQ)
        wo_c = np.ascontiguousarray(Wo[hg * GQ:(hg + 1) * GQ, :]).astype(ml_dtypes.bfloat16).reshape(4, 128, D)
        bqk_c = np.concatenate([bqkv[qcols], bqkv[kcols]]).reshape(8, 128).T
        bqk_sw = np.concatenate([bqk_c[64:128], bqk_c[0:64]], axis=0)
        bv_c = bqkv[vcols].reshape(1, GQ)
        cosT, sin_rot = tabs[b]
        in_maps.append({
            "xt": xts[b], "wqk": wqk_c, "wv": wv_c, "wo": wo_c,
            "bqkt": np.ascontiguousarray(bqk_c), "bqkt_sw": np.ascontiguousarray(bqk_sw),
            "bv": np.ascontiguousarray(bv_c),
            "cost": cosT, "sinrt": sin_rot, "maskd": mask,
            "onesb": onesb,
        })
    return in_maps


def kernel(hidden_states, position_ids, Wqkv, bqkv, Wo, bo, _reps=1):
    bo = np.asarray(bo, dtype=np.float32)
    in_maps = _make_in_maps({
        "hidden_states": hidden_states, "position_ids": position_ids,
        "Wqkv": Wqkv, "bqkv": bqkv, "Wo": Wo, "bo": bo,
    })
    nc = _get_nc(_reps)
    res = run_bass_kernel_spmd(nc, in_maps, core_ids=list(range(NCORES)))

    out = np.empty((B, S, D), dtype=np.float32)
    for b in range(B):
        acc = res.results[b * HG]["outt"].reshape(D, S).astype(np.float32).copy()
        for hg in range(1, HG):
            acc += res.results[b * HG + hg]["outt"].reshape(D, S).astype(np.float32)
        out[b] = acc.T + bo[None, :]
    return out


# revision 11
# speedup vs baseline: 1.0089x; 1.0089x over previous
"""Trainium2 Bass kernel for CustomRoPEAttention (B=2, S=2048, H=16, Dh=128).

Sharding: 8 cores = 2 batches x 4 head-groups (4 heads/core). Each core:
QKV projection (bf16 matmuls) + RoPE for its heads, transposed-layout causal
attention (scores computed as S^T with keys on partitions so the attention
probabilities feed A@V directly as the moving operand -- no PE transposes),
softmax denominators via ap-size-1 matmuls + deferred normalization, and a
partial (transposed) output projection. Host sums the 4 partials per batch.

Self-contained: hardcodes shapes from the problem spec.
"""
import math
from contextlib import ExitStack

import numpy as np
import ml_dtypes

import concourse.mybir as mybir
import concourse.tile as tile
from concourse import bacc
from concourse.bass_utils import run_bass_kernel_spmd
from concourse.masks import make_identity

S = 2048            # sequence
D = 2048            # hidden
NH = 16             # total heads
DH = 128            # head dim
HG = 4              # heads per core
GQ = HG * DH        # 512: per-core q/k/v feature width
B = 2
NCORES = 8
ROPE_THETA = 10000.0
SCALE = 1.0 / math.sqrt(DH)
NEG = -1.0e9
SLAB = 512          # phase-1 sequence slab width
F32 = mybir.dt.float32
BF16 = mybir.dt.bfloat16
F16 = mybir.dt.float16
MULT = mybir.AluOpType.mult
ADD = mybir.AluOpType.add
NB = S // 128       # 16 k/q blocks


def build_nc(reps=1, knobs=None):
    kn = {"p1ps": 3, "p1vps": 2, "p1x": 2, "sps": 2, "avps": 2, "p3ps": 4}
    if knobs:
        kn.update(knobs)
    nc = bacc.Bacc(None, target_bir_lowering=False)
    xt = nc.dram_tensor("xt", [16, 128, S], BF16, kind="ExternalInput")
    wqk = nc.dram_tensor("wqk", [16, 128, 2 * GQ], BF16, kind="ExternalInput")
    wv = nc.dram_tensor("wv", [16, 128, GQ], BF16, kind="ExternalInput")
    wo = nc.dram_tensor("wo", [4, 128, D], BF16, kind="ExternalInput")
    bqkt = nc.dram_tensor("bqkt", [128, 8], F32, kind="ExternalInput")
    bqkt_sw = nc.dram_tensor("bqkt_sw", [128, 8], F32, kind="ExternalInput")
    bv = nc.dram_tensor("bv", [1, GQ], F32, kind="ExternalInput")
    cost = nc.dram_tensor("cost", [128, S], BF16, kind="ExternalInput")    # cos^T
    sinrt = nc.dram_tensor("sinrt", [128, S], BF16, kind="ExternalInput")  # sin^T, rot sign
    maskd = nc.dram_tensor("maskd", [128, 128], F32, kind="ExternalInput")  # tril(-1) NEG
    onesb = nc.dram_tensor("onesb", [128, 1], BF16, kind="ExternalInput")
    outt = nc.dram_tensor("outt", [16, 128, S], BF16, kind="ExternalOutput")
    lrt = nc.dram_tensor("lrt", [HG, 1, 16, 128], F32)  # recip bounce: [16,128] -> [1,2048]

    with tile.TileContext(nc) as tc, ExitStack() as top:
        g = top.enter_context(tc.tile_pool(name="glob", bufs=1))
        tcos = g.tile([128, S], BF16)
        nc.sync.dma_start(out=tcos, in_=cost[:])
        tsin = g.tile([128, S], BF16)
        nc.sync.dma_start(out=tsin, in_=sinrt[:])
        tmask = g.tile([128, 128], F32)
        nc.sync.dma_start(out=tmask, in_=maskd[:])
        ident_f = g.tile([128, 128], F32)
        make_identity(nc, ident_f[:])
        tbqkt = g.tile([128, 8], F32)
        nc.sync.dma_start(out=tbqkt, in_=bqkt[:])
        tbqkt_sw = g.tile([128, 8], F32)
        nc.sync.dma_start(out=tbqkt_sw, in_=bqkt_sw[:])
        tbvb = g.tile([128, GQ], F32)
        nc.sync.dma_start(out=tbvb, in_=bv[:].to_broadcast((128, GQ)))
        tones = g.tile([128, 1], BF16)
        nc.sync.dma_start(out=tones, in_=onesb[:])

        # Whole-kernel residents
        res = top.enter_context(tc.tile_pool(name="res", bufs=1))
        qt = []  # mt 0..3 = Q^T heads, 4..7 = K^T heads, each [128(dh), S] bf16
        for mt in range(2 * HG):
            qt.append(res.tile([128, S], BF16, tag=f"qt{mt}", name=f"qt{mt}"))
        vres = []  # 16 V k-block tiles [128(seq), GQ] bf16
        for t in range(NB):
            vres.append(res.tile([128, GQ], BF16, tag=f"v{t}", name=f"v{t}"))
        two = res.tile([128, 4, D], BF16, tag="two")
        nc.sync.dma_start(out=two, in_=wo.rearrange("kc p f -> p kc f"))
        ct_sb = {}
        for h in range(HG):
            for gq in range(4):
                ct_sb[(h, gq)] = res.tile([128, 512], BF16, tag=f"ct_{h}_{gq}", name=f"ct_{h}_{gq}")

        for _rep in range(reps):
          # ---------------- Phase 1: QKV^T projection + RoPE (all resident) ----------
          with tc.tile_pool(name="p1w", bufs=1) as p1w, \
               tc.tile_pool(name="p1x", bufs=kn["p1x"]) as p1x, \
               tc.tile_pool(name="qswp", bufs=2) as qswp, \
               tc.tile_pool(name="p1stg", bufs=1) as p1stg, \
               tc.tile_pool(name="p1ps", bufs=kn["p1ps"], space="PSUM") as p1ps, \
               tc.tile_pool(name="p1vps", bufs=kn["p1vps"], space="PSUM") as p1vps:
            twqk = []
            twv = []
            for kc in range(16):
                wqkt = p1w.tile([128, 2 * GQ], BF16, tag=f"wqk{kc}")
                nc.sync.dma_start(out=wqkt, in_=wqk[kc])
                twqk.append(wqkt)
                wvt = p1w.tile([128, GQ], BF16, tag=f"wv{kc}")
                nc.sync.dma_start(out=wvt, in_=wv[kc])
                twv.append(wvt)
            for ns in range(S // SLAB):
                sl = slice(ns * SLAB, (ns + 1) * SLAB)
                xs = p1x.tile([128, 16, SLAB], BF16, tag="xs")
                nc.sync.dma_start(out=xs, in_=xt[:, :, sl].rearrange("kc p s -> p kc s"))
                for mt in range(2 * HG):
                    pqk = p1ps.tile([128, SLAB], F32, tag="qkps")
                    for kc in range(16):
                        nc.tensor.matmul(pqk[:], twqk[kc][:, mt * 128:(mt + 1) * 128],
                                         xs[:, kc, :], start=(kc == 0), stop=(kc == 15))
                    nc.scalar.copy(out=qt[mt][:, sl], in_=pqk[:])
                for st in range(SLAB // 128):
                    pv = p1vps.tile([128, GQ], F32, tag="vps")
                    s0 = st * 128
                    for kc in range(16):
                        nc.tensor.matmul(pv[:], xs[:, kc, s0:s0 + 128],
                                         twv[kc][:], start=(kc == 0), stop=(kc == 15))
                    nc.vector.tensor_tensor(out=vres[ns * (SLAB // 128) + st],
                                            in0=pv[:], in1=tbvb[:], op=ADD)
            # RoPE per head tile (full width): q' = (q+b)*cos + swap(q+b)*sin_rot
            for mt in range(2 * HG):
                qsw = qswp.tile([128, S], BF16, tag="qsw")
                nc.sync.dma_start(out=qsw[0:64, :], in_=qt[mt][64:128, :])
                nc.sync.dma_start(out=qsw[64:128, :], in_=qt[mt][0:64, :])
                m1 = p1stg.tile([128, S], BF16, tag="m1")
                nc.vector.scalar_tensor_tensor(
                    out=m1[:], in0=qt[mt][:], scalar=tbqkt[:, mt:mt + 1],
                    in1=tcos[:], op0=ADD, op1=MULT)
                m2 = p1stg.tile([128, S], BF16, tag="m2")
                nc.vector.scalar_tensor_tensor(
                    out=m2[:], in0=qsw[:], scalar=tbqkt_sw[:, mt:mt + 1],
                    in1=tsin[:], op0=ADD, op1=MULT)
                nc.vector.tensor_tensor(out=qt[mt][:], in0=m1[:], in1=m2[:], op=ADD)

          # ---------------- Phase 2: transposed attention ----------------
          p2stack = ExitStack()
          expp = p2stack.enter_context(tc.tile_pool(name="expp", bufs=2))
          lrp = p2stack.enter_context(tc.tile_pool(name="lrp", bufs=2))
          avps = p2stack.enter_context(
              tc.tile_pool(name="avps", bufs=kn["avps"], space="PSUM"))
          rbp = p2stack.enter_context(tc.tile_pool(name="rbp", bufs=2))
          p2inner = ExitStack()
          sps = p2inner.enter_context(
              tc.tile_pool(name="sps", bufs=kn["sps"], space="PSUM"))
          smps = p2inner.enter_context(tc.tile_pool(name="smps", bufs=1, space="PSUM"))

          expT = [None] * HG  # per live head: list of 16 exp(S^T) tiles
          recrow = [None] * HG

          def denom(h, b):
              # ell[q] for q-block b: sum_k exp tiles, ap-1 matmuls, then recip
              lp = smps.tile([128, 1], F32, tag="lp")
              for j in range(b + 1):
                  nc.tensor.matmul(lp[:], expT[h][j][:, (b - j) * 128:(b - j + 1) * 128],
                                   tones[:], start=(j == 0), stop=(j == b))
              nc.vector.reciprocal(out=lrec_cur[h][:, b:b + 1], in_=lp[:])

          def sweep2_group(h, gq):
              # normalize+accumulate: ct = (sum_k V^T[k] expS^T[k]) * recip(ell)
              rbs = rbp.tile([128, 512], F32, tag="rbs")
              nc.gpsimd.partition_broadcast(
                  rbs[:], recrow[h][:, 4 * gq:4 * gq + 4, :])
              ct = avps.tile([128, 512], F32, tag="ct")
              last = 4 * gq + 3
              for j in range(last + 1):
                  if j <= 4 * gq:
                      nc.tensor.matmul(ct[:], vres[j][:, h * 128:(h + 1) * 128],
                                       expT[h][j][:, (4 * gq - j) * 128:(4 * gq - j) * 128 + 512],
                                       start=(j == 0), stop=(j == last))
                  else:
                      w = (4 * gq + 4 - j) * 128
                      nc.tensor.matmul(ct[:, 512 - w:512], vres[j][:, h * 128:(h + 1) * 128],
                                       expT[h][j][:, 0:w], start=False, stop=(j == last))
              nc.vector.tensor_tensor(out=ct_sb[(h, gq)][:], in0=ct[:], in1=rbs[:], op=MULT)

          lrec_cur = {}
          for h in range(HG):
              expT[h] = []
              lrec_cur[h] = lrp.tile([128, 16], F32, tag="lrec", name="lrec")
              recrow[h] = lrp.tile([1, 16, 128], F32, tag="recrow", name="recrow")
              for i in range(NB):
                  w = (NB - i) * 128
                  ex = expp.tile([128, w], BF16, tag=f"expT{i}", name=f"expT{i}")
                  expT[h].append(ex)
                  for c0 in range(0, w, 1024):
                      cw = min(1024, w - c0)
                      sp = sps.tile([128, 1024], F32, tag="sp")
                      for s5 in range(0, cw, 512):
                          w5 = min(512, cw - s5)
                          q0 = i * 128 + c0 + s5
                          nc.tensor.matmul(sp[:, s5:s5 + w5],
                                           qt[HG + h][:, i * 128:(i + 1) * 128],
                                           qt[h][:, q0:q0 + w5], start=True, stop=True)
                      if c0 == 0:
                          nc.vector.tensor_tensor(out=sp[:, 0:128], in0=sp[:, 0:128],
                                                  in1=tmask[:], op=ADD)
                      nc.scalar.activation(out=ex[:, c0:c0 + cw], in_=sp[:, 0:cw],
                                           func=mybir.ActivationFunctionType.Exp,
                                           scale=SCALE)
                  if i >= 2:
                      denom(h, i - 2)
                  if h >= 1 and i % 4 == 3:
                      sweep2_group(h - 1, i // 4)
              denom(h, NB - 2)
              denom(h, NB - 1)
              # recip row: [128,16] -> transpose -> [16,128] -> DRAM -> [1,2048]
              rt = smps.tile([16, 128], F32, tag="rt")
              nc.tensor.transpose(rt[:], lrec_cur[h][:], ident_f[:])
              rts = lrp.tile([16, 128], F32, tag="rts")
              nc.vector.tensor_copy(out=rts[:], in_=rt[:])
              nc.sync.dma_start(out=lrt[h, 0], in_=rts[:])
              nc.sync.dma_start(out=recrow[h][:], in_=lrt[h])

          # close S^T/denom psum pools before opening phase-3 psum
          p2inner.close()

          # ---------------- Phase 3: output projection, interleaving head-3 sweep2 ----
          with tc.tile_pool(name="p3s", bufs=4) as p3s, \
               tc.tile_pool(name="p3ps", bufs=kn["p3ps"], space="PSUM") as p3ps:
              for gq in range(4):
                  sweep2_group(HG - 1, gq)
                  for mt in range(16):
                      op = p3ps.tile([128, 512], F32, tag="op")
                      for kh in range(HG):
                          nc.tensor.matmul(op[:], two[:, kh, mt * 128:(mt + 1) * 128],
                                           ct_sb[(kh, gq)][:],
                                           start=(kh == 0), stop=(kh == 3))
                      ob = p3s.tile([128, 512], BF16, tag="ob")
                      if mt % 2 == 0:
                          nc.vector.tensor_copy(out=ob[:], in_=op[:])
                      else:
                          nc.scalar.copy(out=ob[:], in_=op[:])
                      nc.sync.dma_start(out=outt[mt, :, gq * 512:(gq + 1) * 512], in_=ob[:])
          p2stack.close()
    nc.finalize()
    return nc


_NC_CACHE = {}


def _get_nc(reps=1):
    if reps not in _NC_CACHE:
        _NC_CACHE[reps] = build_nc(reps)
    return _NC_CACHE[reps]


def _rope_tables(position_ids_b):
    pos = position_ids_b.astype(np.float32)
    inv_freq = (1.0 / (ROPE_THETA ** (np.arange(0, DH, 2, dtype=np.float32) / np.float32(DH))))
    ang = pos[:, None] * inv_freq[None, :]          # [S, 64]
    emb = np.concatenate([ang, ang], axis=-1)       # [S, 128]
    cosT = np.ascontiguousarray(np.cos(emb).T)      # [128, S]
    sinT = np.sin(emb).T
    sin_rot = np.concatenate([-sinT[0:64], sinT[64:128]], axis=0)
    return cosT.astype(ml_dtypes.bfloat16), np.ascontiguousarray(sin_rot).astype(ml_dtypes.bfloat16)


def _make_in_maps(inputs):
    hidden_states = np.asarray(inputs["hidden_states"], dtype=np.float32)
    position_ids = np.asarray(inputs["position_ids"])
    Wqkv = np.asarray(inputs["Wqkv"], dtype=np.float32)
    bqkv = np.asarray(inputs["bqkv"], dtype=np.float32)
    Wo = np.asarray(inputs["Wo"], dtype=np.float32)

    mask = np.tril(np.full((128, 128), NEG, dtype=np.float32), k=-1)
    tabs = [_rope_tables(np.asarray(position_ids)[b]) for b in range(B)]
    xts = [np.ascontiguousarray(hidden_states[b].T).astype(ml_dtypes.bfloat16)
           .reshape(16, 128, S) for b in range(B)]
    onesb = np.ones((128, 1), dtype=ml_dtypes.bfloat16)

    in_maps = []
    for c in range(NCORES):
        b, hg = divmod(c, HG)
        qcols = slice(hg * GQ, (hg + 1) * GQ)
        kcols = slice(D + hg * GQ, D + (hg + 1) * GQ)
        vcols = slice(2 * D + hg * GQ, 2 * D + (hg + 1) * GQ)
        wqk_c = np.ascontiguousarray(
            np.concatenate([Wqkv[:, qcols], Wqkv[:, kcols]], axis=1)
        ).astype(ml_dtypes.bfloat16).reshape(16, 128, 2 * GQ)
        wv_c = np.ascontiguousarray(Wqkv[:, vcols]).astype(ml_dtypes.bfloat16).reshape(16, 128, GQ)
        wo_c = np.ascontiguousarray(Wo[hg * GQ:(hg + 1) * GQ, :]).astype(ml_dtypes.bfloat16).reshape(4, 128, D)
        bqk_c = np.concatenate([bqkv[qcols], bqkv[kcols]]).reshape(8, 128).T
        bqk_sw = np.concatenate([bqk_c[64:128], bqk_c[0:64]], axis=0)
        bv_c = bqkv[vcols].reshape(1, GQ)
        cosT, sin_rot = tabs[b]
        in_maps.append({
            "xt": xts[b], "wqk": wqk_c, "wv": wv_c, "wo": wo_c,
            "bqkt": np.ascontiguousarray(bqk_c), "bqkt_sw": np.ascontiguousarray(bqk_sw),
            "bv": np.ascontiguousarray(bv_c),
            "cost": cosT, "sinrt": sin_rot, "maskd": mask,
            "onesb": onesb,
        })
    return in_maps


def kernel(hidden_states, position_ids, Wqkv, bqkv, Wo, bo, _reps=1):
    bo = np.asarray(bo, dtype=np.float32)
    in_maps = _make_in_maps({
        "hidden_states": hidden_states, "position_ids": position_ids,
        "Wqkv": Wqkv, "bqkv": bqkv, "Wo": Wo, "bo": bo,
    })
    nc = _get_nc(_reps)
    res = run_bass_kernel_spmd(nc, in_maps, core_ids=list(range(NCORES)))

    out = np.empty((B, S, D), dtype=np.float32)
    for b in range(B):
        acc = res.results[b * HG]["outt"].reshape(D, S).astype(np.float32).copy()
        for hg in range(1, HG):
            acc += res.results[b * HG + hg]["outt"].reshape(D, S).astype(np.float32)
        out[b] = acc.T + bo[None, :]
    return out


# revision 12
# speedup vs baseline: 1.1019x; 1.0922x over previous
"""Trainium2 Bass kernel for CustomRoPEAttention (B=2, S=2048, H=16, Dh=128).

Sharding: 8 cores = 2 batches x 4 head-groups (4 heads/core). Each core:
QKV projection (bf16 matmuls) + RoPE for its heads, transposed-layout causal
attention (scores computed as S^T with keys on partitions so the attention
probabilities feed A@V directly as the moving operand -- no PE transposes),
softmax denominators via ap-size-1 matmuls + deferred normalization, and a
partial (transposed) output projection. Host sums the 4 partials per batch.

Self-contained: hardcodes shapes from the problem spec.
"""
import math
from contextlib import ExitStack

import numpy as np
import ml_dtypes

import concourse.mybir as mybir
import concourse.tile as tile
from concourse import bacc
from concourse.bass_utils import run_bass_kernel_spmd
from concourse.masks import make_identity

S = 2048            # sequence
D = 2048            # hidden
NH = 16             # total heads
DH = 128            # head dim
HG = 4              # heads per core
GQ = HG * DH        # 512: per-core q/k/v feature width
B = 2
NCORES = 8
ROPE_THETA = 10000.0
SCALE = 1.0 / math.sqrt(DH)
NEG = -1.0e9
SLAB = 512          # phase-1 sequence slab width
F32 = mybir.dt.float32
BF16 = mybir.dt.bfloat16
F16 = mybir.dt.float16
MULT = mybir.AluOpType.mult
ADD = mybir.AluOpType.add
NB = S // 128       # 16 k/q blocks


def build_nc(reps=1, knobs=None):
    kn = {"p1ps": 3, "p1vps": 2, "p1x": 2, "sps": 2, "avps": 2, "p3ps": 4}
    if knobs:
        kn.update(knobs)
    nc = bacc.Bacc(None, target_bir_lowering=False)
    xt = nc.dram_tensor("xt", [16, 128, S], BF16, kind="ExternalInput")
    wqk = nc.dram_tensor("wqk", [16, 128, 2 * GQ], BF16, kind="ExternalInput")
    wv = nc.dram_tensor("wv", [16, 128, GQ], BF16, kind="ExternalInput")
    wo = nc.dram_tensor("wo", [4, 128, D], BF16, kind="ExternalInput")
    bqkt = nc.dram_tensor("bqkt", [128, 8], F32, kind="ExternalInput")
    bv = nc.dram_tensor("bv", [1, GQ], F32, kind="ExternalInput")
    cost = nc.dram_tensor("cost", [128, S], BF16, kind="ExternalInput")    # cos^T
    sinrt = nc.dram_tensor("sinrt", [128, S], BF16, kind="ExternalInput")  # sin^T, rot sign
    maskd = nc.dram_tensor("maskd", [128, 128], F32, kind="ExternalInput")  # tril(-1) NEG
    onesb = nc.dram_tensor("onesb", [128, 1], BF16, kind="ExternalInput")
    outt = nc.dram_tensor("outt", [16, 128, S], BF16, kind="ExternalOutput")
    lrt = nc.dram_tensor("lrt", [HG, 1, 16, 128], F32)  # recip bounce: [16,128] -> [1,2048]

    with tile.TileContext(nc) as tc, ExitStack() as top:
        g = top.enter_context(tc.tile_pool(name="glob", bufs=1))
        tcos = g.tile([128, S], BF16)
        tsin = g.tile([128, S], BF16)
        tmask = g.tile([128, 128], F32)
        ident_f = g.tile([128, 128], F32)
        make_identity(nc, ident_f[:])
        tbqkt = g.tile([128, 8], F32)
        nc.sync.dma_start(out=tbqkt, in_=bqkt[:])
        tbvb = g.tile([128, GQ], F32)
        tones = g.tile([128, 1], BF16)

        def load_consts():
            nc.sync.dma_start(out=tcos, in_=cost[:])
            nc.sync.dma_start(out=tsin, in_=sinrt[:])
            nc.sync.dma_start(out=tmask, in_=maskd[:])
            nc.sync.dma_start(out=tbvb, in_=bv[:].to_broadcast((128, GQ)))
            nc.sync.dma_start(out=tones, in_=onesb[:])

        # Whole-kernel residents
        res = top.enter_context(tc.tile_pool(name="res", bufs=1))
        qt = []  # mt 0..3 = Q^T heads, 4..7 = K^T heads, each [128(dh), S] bf16
        for mt in range(2 * HG):
            qt.append(res.tile([128, S], BF16, tag=f"qt{mt}", name=f"qt{mt}"))
        vres = []  # 16 V k-block tiles [128(seq), GQ] bf16
        for t in range(NB):
            vres.append(res.tile([128, GQ], BF16, tag=f"v{t}", name=f"v{t}"))
        two = res.tile([128, 4, D], BF16, tag="two")
        ct_sb = {}
        for h in range(HG):
            for gq in range(4):
                ct_sb[(h, gq)] = res.tile([128, 512], BF16, tag=f"ct_{h}_{gq}", name=f"ct_{h}_{gq}")

        for _rep in range(reps):
          # ---------------- Phase 1: QKV^T projection + RoPE (all resident) ----------
          with tc.tile_pool(name="p1w", bufs=1) as p1w, \
               tc.tile_pool(name="p1x", bufs=kn["p1x"]) as p1x, \
               tc.tile_pool(name="qswp", bufs=2) as qswp, \
               tc.tile_pool(name="p1stg", bufs=1) as p1stg, \
               tc.tile_pool(name="p1ps", bufs=kn["p1ps"], space="PSUM") as p1ps, \
               tc.tile_pool(name="p1vps", bufs=kn["p1vps"], space="PSUM") as p1vps:
            # startup-critical order: xs0 (per-kc so matmul 0 starts fast),
            # wqk stream, xs1, wv stream, consts
            xs0 = p1x.tile([128, 16, SLAB], BF16, tag="xs", name="xs0")
            for kc in range(16):
                nc.sync.dma_start(out=xs0[:, kc, :], in_=xt[kc, :, 0:SLAB])
            twqk = []
            twv = []
            for kc in range(16):
                wqkt = p1w.tile([128, 2 * GQ], BF16, tag=f"wqk{kc}")
                nc.sync.dma_start(out=wqkt, in_=wqk[kc])
                twqk.append(wqkt)
            xs1 = p1x.tile([128, 16, SLAB], BF16, tag="xs", name="xs1")
            nc.sync.dma_start(out=xs1, in_=xt[:, :, SLAB:2 * SLAB].rearrange("kc p s -> p kc s"))
            for kc in range(16):
                wvt = p1w.tile([128, GQ], BF16, tag=f"wv{kc}")
                nc.sync.dma_start(out=wvt, in_=wv[kc])
                twv.append(wvt)
            load_consts()
            xs_pre = {0: xs0, 1: xs1}
            for ns in range(S // SLAB):
                sl = slice(ns * SLAB, (ns + 1) * SLAB)
                if ns in xs_pre:
                    xs = xs_pre[ns]
                else:
                    xs = p1x.tile([128, 16, SLAB], BF16, tag="xs", name=f"xs{ns}")
                    nc.sync.dma_start(out=xs, in_=xt[:, :, sl].rearrange("kc p s -> p kc s"))
                for mt in range(2 * HG):
                    pqk = p1ps.tile([128, SLAB], F32, tag="qkps")
                    for kc in range(16):
                        nc.tensor.matmul(pqk[:], twqk[kc][:, mt * 128:(mt + 1) * 128],
                                         xs[:, kc, :], start=(kc == 0), stop=(kc == 15))
                    nc.scalar.activation(out=qt[mt][:, sl], in_=pqk[:],
                                         func=mybir.ActivationFunctionType.Identity,
                                         bias=tbqkt[:, mt:mt + 1])
                for st in range(SLAB // 128):
                    pv = p1vps.tile([128, GQ], F32, tag="vps")
                    s0 = st * 128
                    for kc in range(16):
                        nc.tensor.matmul(pv[:], xs[:, kc, s0:s0 + 128],
                                         twv[kc][:], start=(kc == 0), stop=(kc == 15))
                    nc.vector.tensor_tensor(out=vres[ns * (SLAB // 128) + st],
                                            in0=pv[:], in1=tbvb[:], op=ADD)
            # RoPE per head tile (full width): q' = qb*cos + swap(qb)*sin_rot
            # (bias already applied in the psum copy); head 0's q,k first
            for mt in (0, 4, 1, 5, 2, 6, 3, 7):
                qsw = qswp.tile([128, S], BF16, tag="qsw")
                nc.sync.dma_start(out=qsw[0:64, :], in_=qt[mt][64:128, :])
                nc.sync.dma_start(out=qsw[64:128, :], in_=qt[mt][0:64, :])
                m1 = p1stg.tile([128, S], BF16, tag="m1")
                nc.vector.tensor_tensor(out=m1[:], in0=qt[mt][:], in1=tcos[:], op=MULT)
                m2 = p1stg.tile([128, S], BF16, tag="m2")
                nc.vector.tensor_tensor(out=m2[:], in0=qsw[:], in1=tsin[:], op=MULT)
                nc.vector.tensor_tensor(out=qt[mt][:], in0=m1[:], in1=m2[:], op=ADD)

          # ---------------- Phase 2: transposed attention ----------------
          p2stack = ExitStack()
          expp = p2stack.enter_context(tc.tile_pool(name="expp", bufs=2))
          lrp = p2stack.enter_context(tc.tile_pool(name="lrp", bufs=2))
          avps = p2stack.enter_context(
              tc.tile_pool(name="avps", bufs=kn["avps"], space="PSUM"))
          rbp = p2stack.enter_context(tc.tile_pool(name="rbp", bufs=2))
          p2inner = ExitStack()
          sps = p2inner.enter_context(
              tc.tile_pool(name="sps", bufs=kn["sps"], space="PSUM"))
          smps = p2inner.enter_context(tc.tile_pool(name="smps", bufs=1, space="PSUM"))

          nc.sync.dma_start(out=two, in_=wo.rearrange("kc p f -> p kc f"))
          expT = [None] * HG  # per live head: list of 16 exp(S^T) tiles
          recrow = [None] * HG

          def denom(h, b):
              # ell[q] for q-block b: sum_k exp tiles, ap-1 matmuls, then recip
              lp = smps.tile([128, 1], F32, tag="lp")
              for j in range(b + 1):
                  nc.tensor.matmul(lp[:], expT[h][j][:, (b - j) * 128:(b - j + 1) * 128],
                                   tones[:], start=(j == 0), stop=(j == b))
              nc.vector.reciprocal(out=lrec_cur[h][:, b:b + 1], in_=lp[:])

          def sweep2_group(h, gq):
              # normalize+accumulate: ct = (sum_k V^T[k] expS^T[k]) * recip(ell)
              rbs = rbp.tile([128, 512], F32, tag="rbs")
              nc.gpsimd.partition_broadcast(
                  rbs[:], recrow[h][:, 4 * gq:4 * gq + 4, :])
              ct = avps.tile([128, 512], F32, tag="ct")
              last = 4 * gq + 3
              for j in range(last + 1):
                  if j <= 4 * gq:
                      nc.tensor.matmul(ct[:], vres[j][:, h * 128:(h + 1) * 128],
                                       expT[h][j][:, (4 * gq - j) * 128:(4 * gq - j) * 128 + 512],
                                       start=(j == 0), stop=(j == last))
                  else:
                      w = (4 * gq + 4 - j) * 128
                      nc.tensor.matmul(ct[:, 512 - w:512], vres[j][:, h * 128:(h + 1) * 128],
                                       expT[h][j][:, 0:w], start=False, stop=(j == last))
              nc.vector.tensor_tensor(out=ct_sb[(h, gq)][:], in0=ct[:], in1=rbs[:], op=MULT)

          lrec_cur = {}
          for h in range(HG):
              expT[h] = []
              lrec_cur[h] = lrp.tile([128, 16], F32, tag="lrec", name="lrec")
              recrow[h] = lrp.tile([1, 16, 128], F32, tag="recrow", name="recrow")
              for i in range(NB):
                  w = (NB - i) * 128
                  ex = expp.tile([128, w], BF16, tag=f"expT{i}", name=f"expT{i}")
                  expT[h].append(ex)
                  for c0 in range(0, w, 1024):
                      cw = min(1024, w - c0)
                      sp = sps.tile([128, 1024], F32, tag="sp")
                      for s5 in range(0, cw, 512):
                          w5 = min(512, cw - s5)
                          q0 = i * 128 + c0 + s5
                          nc.tensor.matmul(sp[:, s5:s5 + w5],
                                           qt[HG + h][:, i * 128:(i + 1) * 128],
                                           qt[h][:, q0:q0 + w5], start=True, stop=True)
                      if c0 == 0:
                          nc.vector.tensor_tensor(out=sp[:, 0:128], in0=sp[:, 0:128],
                                                  in1=tmask[:], op=ADD)
                      nc.scalar.activation(out=ex[:, c0:c0 + cw], in_=sp[:, 0:cw],
                                           func=mybir.ActivationFunctionType.Exp,
                                           scale=SCALE)
                  if i >= 2:
                      denom(h, i - 2)
                  if h >= 1 and i % 4 == 3:
                      sweep2_group(h - 1, i // 4)
              denom(h, NB - 2)
              denom(h, NB - 1)
              # recip row: [128,16] -> transpose -> [16,128] -> DRAM -> [1,2048]
              rt = smps.tile([16, 128], F32, tag="rt")
              nc.tensor.transpose(rt[:], lrec_cur[h][:], ident_f[:])
              rts = lrp.tile([16, 128], F32, tag="rts")
              nc.vector.tensor_copy(out=rts[:], in_=rt[:])
              nc.sync.dma_start(out=lrt[h, 0], in_=rts[:])
              nc.sync.dma_start(out=recrow[h][:], in_=lrt[h])

          # close S^T/denom psum pools before opening phase-3 psum
          p2inner.close()

          # ---------------- Phase 3: output projection, interleaving head-3 sweep2 ----
          with tc.tile_pool(name="p3s", bufs=4) as p3s, \
               tc.tile_pool(name="p3ps", bufs=kn["p3ps"], space="PSUM") as p3ps:
              for gq in range(4):
                  sweep2_group(HG - 1, gq)
                  for mt in range(16):
                      op = p3ps.tile([128, 512], F32, tag="op")
                      for kh in range(HG):
                          nc.tensor.matmul(op[:], two[:, kh, mt * 128:(mt + 1) * 128],
                                           ct_sb[(kh, gq)][:],
                                           start=(kh == 0), stop=(kh == 3))
                      ob = p3s.tile([128, 512], BF16, tag="ob")
                      if mt % 2 == 0:
                          nc.vector.tensor_copy(out=ob[:], in_=op[:])
                      else:
                          nc.scalar.copy(out=ob[:], in_=op[:])
                      nc.sync.dma_start(out=outt[mt, :, gq * 512:(gq + 1) * 512], in_=ob[:])
          p2stack.close()
    nc.finalize()
    return nc


_NC_CACHE = {}


def _get_nc(reps=1):
    if reps not in _NC_CACHE:
        _NC_CACHE[reps] = build_nc(reps)
    return _NC_CACHE[reps]


def _rope_tables(position_ids_b):
    pos = position_ids_b.astype(np.float32)
    inv_freq = (1.0 / (ROPE_THETA ** (np.arange(0, DH, 2, dtype=np.float32) / np.float32(DH))))
    ang = pos[:, None] * inv_freq[None, :]          # [S, 64]
    emb = np.concatenate([ang, ang], axis=-1)       # [S, 128]
    cosT = np.ascontiguousarray(np.cos(emb).T)      # [128, S]
    sinT = np.sin(emb).T
    sin_rot = np.concatenate([-sinT[0:64], sinT[64:128]], axis=0)
    return cosT.astype(ml_dtypes.bfloat16), np.ascontiguousarray(sin_rot).astype(ml_dtypes.bfloat16)


def _make_in_maps(inputs):
    hidden_states = np.asarray(inputs["hidden_states"], dtype=np.float32)
    position_ids = np.asarray(inputs["position_ids"])
    Wqkv = np.asarray(inputs["Wqkv"], dtype=np.float32)
    bqkv = np.asarray(inputs["bqkv"], dtype=np.float32)
    Wo = np.asarray(inputs["Wo"], dtype=np.float32)

    mask = np.tril(np.full((128, 128), NEG, dtype=np.float32), k=-1)
    tabs = [_rope_tables(np.asarray(position_ids)[b]) for b in range(B)]
    xts = [np.ascontiguousarray(hidden_states[b].T).astype(ml_dtypes.bfloat16)
           .reshape(16, 128, S) for b in range(B)]
    onesb = np.ones((128, 1), dtype=ml_dtypes.bfloat16)

    in_maps = []
    for c in range(NCORES):
        b, hg = divmod(c, HG)
        qcols = slice(hg * GQ, (hg + 1) * GQ)
        kcols = slice(D + hg * GQ, D + (hg + 1) * GQ)
        vcols = slice(2 * D + hg * GQ, 2 * D + (hg + 1) * GQ)
        wqk_c = np.ascontiguousarray(
            np.concatenate([Wqkv[:, qcols], Wqkv[:, kcols]], axis=1)
        ).astype(ml_dtypes.bfloat16).reshape(16, 128, 2 * GQ)
        wv_c = np.ascontiguousarray(Wqkv[:, vcols]).astype(ml_dtypes.bfloat16).reshape(16, 128, GQ)
        wo_c = np.ascontiguousarray(Wo[hg * GQ:(hg + 1) * GQ, :]).astype(ml_dtypes.bfloat16).reshape(4, 128, D)
        bqk_c = np.concatenate([bqkv[qcols], bqkv[kcols]]).reshape(8, 128).T
        bv_c = bqkv[vcols].reshape(1, GQ)
        cosT, sin_rot = tabs[b]
        in_maps.append({
            "xt": xts[b], "wqk": wqk_c, "wv": wv_c, "wo": wo_c,
            "bqkt": np.ascontiguousarray(bqk_c),
            "bv": np.ascontiguousarray(bv_c),
            "cost": cosT, "sinrt": sin_rot, "maskd": mask,
            "onesb": onesb,
        })
    return in_maps


def kernel(hidden_states, position_ids, Wqkv, bqkv, Wo, bo, _reps=1):
    bo = np.asarray(bo, dtype=np.float32)
    in_maps = _make_in_maps({
        "hidden_states": hidden_states, "position_ids": position_ids,
        "Wqkv": Wqkv, "bqkv": bqkv, "Wo": Wo, "bo": bo,
    })
    nc = _get_nc(_reps)
    res = run_bass_kernel_spmd(nc, in_maps, core_ids=list(range(NCORES)))

    out = np.empty((B, S, D), dtype=np.float32)
    for b in range(B):
        acc = res.results[b * HG]["outt"].reshape(D, S).astype(np.float32).copy()
        for hg in range(1, HG):
            acc += res.results[b * HG + hg]["outt"].reshape(D, S).astype(np.float32)
        out[b] = acc.T + bo[None, :]
    return out


# revision 15
# speedup vs baseline: 1.2508x; 1.1352x over previous
"""Trainium2 Bass kernel for CustomRoPEAttention (B=2, S=2048, H=16, Dh=128).

Sharding: 8 cores = 2 batches x 4 head-groups (4 heads/core). Each core:
QKV projection (bf16 matmuls) + RoPE for its heads, transposed-layout causal
attention (scores computed as S^T with keys on partitions so the attention
probabilities feed A@V directly as the moving operand -- no PE transposes),
softmax denominators via ap-size-1 matmuls + deferred normalization, and a
partial (transposed) output projection. Host sums the 4 partials per batch.

Self-contained: hardcodes shapes from the problem spec.
"""
import math
from contextlib import ExitStack

import numpy as np
import ml_dtypes

import concourse.mybir as mybir
import concourse.tile as tile
from concourse import bacc
from concourse.bass_utils import run_bass_kernel_spmd
from concourse.masks import make_identity

S = 2048            # sequence
D = 2048            # hidden
NH = 16             # total heads
DH = 128            # head dim
HG = 4              # heads per core
GQ = HG * DH        # 512: per-core q/k/v feature width
B = 2
NCORES = 8
ROPE_THETA = 10000.0
SCALE = 1.0 / math.sqrt(DH)
NEG = -1.0e9
SLAB = 512          # phase-1 sequence slab width
XSC = 16.0          # fp8 pre-scale for x (keeps hi/lo in e4m3 normal range)
WSC = 512.0         # fp8 pre-scale for Wqkv
INV_SC = 1.0 / (XSC * WSC)
F32 = mybir.dt.float32
BF16 = mybir.dt.bfloat16
F16 = mybir.dt.float16
MULT = mybir.AluOpType.mult
ADD = mybir.AluOpType.add
NB = S // 128       # 16 k/q blocks


def build_nc(reps=1, knobs=None):
    kn = {"p1ps": 3, "p1vps": 2, "p1x": 2, "sps": 2, "avps": 2, "p3ps": 4}
    if knobs:
        kn.update(knobs)
    nc = bacc.Bacc(None, target_bir_lowering=False)
    F8 = mybir.dt.float8e4
    xh = nc.dram_tensor("xh", [128, 8, 2, S], F8, kind="ExternalInput")
    xl = nc.dram_tensor("xl", [128, 8, 2, S], F8, kind="ExternalInput")
    # per-mt packed qk weights: [mt, p, kc2, i, m]
    wqkh = nc.dram_tensor("wqkh", [8, 128, 8, 2, 128], F8, kind="ExternalInput")
    wqkl = nc.dram_tensor("wqkl", [8, 128, 8, 2, 128], F8, kind="ExternalInput")
    wvh = nc.dram_tensor("wvh", [8, 128, 2, GQ], F8, kind="ExternalInput")
    wvl = nc.dram_tensor("wvl", [8, 128, 2, GQ], F8, kind="ExternalInput")
    wo = nc.dram_tensor("wo", [4, 128, D], BF16, kind="ExternalInput")
    bqkt = nc.dram_tensor("bqkt", [128, 8], F32, kind="ExternalInput")
    bv = nc.dram_tensor("bv", [1, GQ], F32, kind="ExternalInput")
    cost = nc.dram_tensor("cost", [128, S], BF16, kind="ExternalInput")    # cos^T
    sinrt = nc.dram_tensor("sinrt", [128, S], BF16, kind="ExternalInput")  # sin^T, rot sign
    maskd = nc.dram_tensor("maskd", [128, 128], F32, kind="ExternalInput")  # tril(-1) NEG
    onesb = nc.dram_tensor("onesb", [128, 1], BF16, kind="ExternalInput")
    outt = nc.dram_tensor("outt", [16, 128, S], BF16, kind="ExternalOutput")
    lrt = nc.dram_tensor("lrt", [HG, 1, 16, 128], F32)  # recip bounce: [16,128] -> [1,2048]

    with tile.TileContext(nc) as tc, ExitStack() as top:
        g = top.enter_context(tc.tile_pool(name="glob", bufs=1))
        tcos = g.tile([128, S], BF16)
        tsin = g.tile([128, S], BF16)
        tmask = g.tile([128, 128], F32)
        ident_f = g.tile([128, 128], F32)
        make_identity(nc, ident_f[:])
        tbqkt = g.tile([128, 8], F32)
        nc.sync.dma_start(out=tbqkt, in_=bqkt[:])
        tbvb = g.tile([128, GQ], F32)
        tones = g.tile([128, 1], BF16)
        tinv = g.tile([128, 1], F32)
        nc.vector.memset(tinv[:], INV_SC)

        def load_consts():
            nc.sync.dma_start(out=tcos, in_=cost[:])
            nc.sync.dma_start(out=tsin, in_=sinrt[:])
            nc.sync.dma_start(out=tmask, in_=maskd[:])
            nc.sync.dma_start(out=tbvb, in_=bv[:].to_broadcast((128, GQ)))
            nc.sync.dma_start(out=tones, in_=onesb[:])

        # Whole-kernel residents
        res = top.enter_context(tc.tile_pool(name="res", bufs=1))
        qt = []  # mt 0..3 = Q^T heads, 4..7 = K^T heads, each [128(dh), S] bf16
        for mt in range(2 * HG):
            qt.append(res.tile([128, S], BF16, tag=f"qt{mt}", name=f"qt{mt}"))
        vres = []  # 16 V k-block tiles [128(seq), GQ] bf16
        for t in range(NB):
            vres.append(res.tile([128, GQ], BF16, tag=f"v{t}", name=f"v{t}"))
        two = res.tile([128, 4, D], BF16, tag="two")
        ct_sb = {}
        for h in range(HG):
            for gq in range(4):
                ct_sb[(h, gq)] = res.tile([128, 512], BF16, tag=f"ct_{h}_{gq}", name=f"ct_{h}_{gq}")

        for _rep in range(reps):
          # ---------------- Phase 1: QKV^T projection + RoPE (all resident) ----------
          with tc.tile_pool(name="p1w", bufs=1) as p1w, \
               tc.tile_pool(name="p1x", bufs=kn["p1x"]) as p1x, \
               tc.tile_pool(name="qswp", bufs=2) as qswp, \
               tc.tile_pool(name="p1stg", bufs=1) as p1stg, \
               tc.tile_pool(name="p1ps", bufs=kn["p1ps"], space="PSUM") as p1ps, \
               tc.tile_pool(name="p1vps", bufs=kn["p1vps"], space="PSUM") as p1vps:
            # startup-critical order: xs0h per-kc2, per-mt wqk stream (h then l),
            # xs0l, xs1, wv, consts
            F8 = mybir.dt.float8e4
            DR = mybir.MatmulPerfMode.DoubleRow
            xs0h = p1x.tile([128, 8, 2, SLAB], F8, tag="xsh", name="xs0h")
            for kc2 in range(8):
                nc.sync.dma_start(out=xs0h[:, kc2, :, :], in_=xh[:, kc2, :, 0:SLAB])
            twqkh, twqkl, twvh, twvl = [], [], [], []
            for mt in range(8):
                wt = p1w.tile([128, 8, 2, 128], F8, tag=f"wqkh{mt}")
                nc.sync.dma_start(out=wt, in_=wqkh[mt])
                twqkh.append(wt)
            xs0l = p1x.tile([128, 8, 2, SLAB], F8, tag="xsl", name="xs0l")
            for kc2 in range(8):
                nc.sync.dma_start(out=xs0l[:, kc2, :, :], in_=xl[:, kc2, :, 0:SLAB])
            for mt in range(8):
                wt = p1w.tile([128, 8, 2, 128], F8, tag=f"wqkl{mt}")
                nc.sync.dma_start(out=wt, in_=wqkl[mt])
                twqkl.append(wt)
            xs1h = p1x.tile([128, 8, 2, SLAB], F8, tag="xsh", name="xs1h")
            nc.sync.dma_start(out=xs1h, in_=xh[:, :, :, SLAB:2 * SLAB])
            xs1l = p1x.tile([128, 8, 2, SLAB], F8, tag="xsl", name="xs1l")
            nc.sync.dma_start(out=xs1l, in_=xl[:, :, :, SLAB:2 * SLAB])
            for kc2 in range(8):
                wt = p1w.tile([128, 2, GQ], F8, tag=f"wvh{kc2}")
                nc.sync.dma_start(out=wt, in_=wvh[kc2])
                twvh.append(wt)
                wt = p1w.tile([128, 2, GQ], F8, tag=f"wvl{kc2}")
                nc.sync.dma_start(out=wt, in_=wvl[kc2])
                twvl.append(wt)
            load_consts()
            xs_pre = {0: (xs0h, xs0l), 1: (xs1h, xs1l)}
            for ns in range(S // SLAB):
                sl = slice(ns * SLAB, (ns + 1) * SLAB)
                if ns in xs_pre:
                    xsh, xsl = xs_pre[ns]
                else:
                    xsh = p1x.tile([128, 8, 2, SLAB], F8, tag="xsh", name=f"xs{ns}h")
                    nc.sync.dma_start(out=xsh, in_=xh[:, :, :, sl])
                    xsl = p1x.tile([128, 8, 2, SLAB], F8, tag="xsl", name=f"xs{ns}l")
                    nc.sync.dma_start(out=xsl, in_=xl[:, :, :, sl])
                for mt in range(2 * HG):
                    pqk = p1ps.tile([128, SLAB], F32, tag="qkps")
                    passes = [(twqkh[mt], xsh), (twqkh[mt], xsl), (twqkl[mt], xsh)]
                    np_ = len(passes)
                    for pi, (wt, xt_) in enumerate(passes):
                        for kc2 in range(8):
                            nc.tensor.matmul(pqk[:], wt[:, kc2, :, :], xt_[:, kc2, :, :],
                                             start=(pi == 0 and kc2 == 0),
                                             stop=(pi == np_ - 1 and kc2 == 7),
                                             perf_mode=DR)
                    nc.scalar.activation(out=qt[mt][:, sl], in_=pqk[:],
                                         func=mybir.ActivationFunctionType.Identity,
                                         scale=INV_SC, bias=tbqkt[:, mt:mt + 1])
                for st in range(SLAB // 128):
                    pv = p1vps.tile([128, GQ], F32, tag="vps")
                    s0 = st * 128
                    passes = [(xsh, twvh), (xsl, twvh), (xsh, twvl)]
                    np_ = len(passes)
                    for pi, (xt_, wv_) in enumerate(passes):
                        for kc2 in range(8):
                            nc.tensor.matmul(pv[:], xt_[:, kc2, :, s0:s0 + 128],
                                             wv_[kc2][:], start=(pi == 0 and kc2 == 0),
                                             stop=(pi == np_ - 1 and kc2 == 7),
                                             perf_mode=DR)
                    nc.vector.scalar_tensor_tensor(
                        out=vres[ns * (SLAB // 128) + st], in0=pv[:],
                        scalar=tinv[:], in1=tbvb[:], op0=MULT, op1=ADD)
            # RoPE per head tile (full width): q' = qb*cos + swap(qb)*sin_rot
            # (bias already applied in the psum copy); head 0's q,k first
            for mt in (0, 4, 1, 5, 2, 6, 3, 7):
                qsw = qswp.tile([128, S], BF16, tag="qsw")
                nc.sync.dma_start(out=qsw[0:64, :], in_=qt[mt][64:128, :])
                nc.sync.dma_start(out=qsw[64:128, :], in_=qt[mt][0:64, :])
                m1 = p1stg.tile([128, S], BF16, tag="m1")
                nc.vector.tensor_tensor(out=m1[:], in0=qt[mt][:], in1=tcos[:], op=MULT)
                m2 = p1stg.tile([128, S], BF16, tag="m2")
                nc.vector.tensor_tensor(out=m2[:], in0=qsw[:], in1=tsin[:], op=MULT)
                nc.vector.tensor_tensor(out=qt[mt][:], in0=m1[:], in1=m2[:], op=ADD)

          # ---------------- Phase 2: transposed attention ----------------
          p2stack = ExitStack()
          expp = p2stack.enter_context(tc.tile_pool(name="expp", bufs=2))
          lrp = p2stack.enter_context(tc.tile_pool(name="lrp", bufs=2))
          avps = p2stack.enter_context(
              tc.tile_pool(name="avps", bufs=kn["avps"], space="PSUM"))
          rbp = p2stack.enter_context(tc.tile_pool(name="rbp", bufs=2))
          p2inner = ExitStack()
          sps = p2inner.enter_context(
              tc.tile_pool(name="sps", bufs=kn["sps"], space="PSUM"))
          smps = p2inner.enter_context(tc.tile_pool(name="smps", bufs=1, space="PSUM"))

          nc.sync.dma_start(out=two, in_=wo.rearrange("kc p f -> p kc f"))
          expT = [None] * HG  # per live head: list of 16 exp(S^T) tiles
          recrow = [None] * HG

          def denom(h, b):
              # ell[q] for q-block b: sum_k exp tiles, ap-1 matmuls, then recip
              lp = smps.tile([128, 1], F32, tag="lp")
              for j in range(b + 1):
                  nc.tensor.matmul(lp[:], expT[h][j][:, (b - j) * 128:(b - j + 1) * 128],
                                   tones[:], start=(j == 0), stop=(j == b))
              nc.vector.reciprocal(out=lrec_cur[h][:, b:b + 1], in_=lp[:])

          def sweep2_group(h, gq):
              # normalize+accumulate: ct = (sum_k V^T[k] expS^T[k]) * recip(ell)
              rbs = rbp.tile([128, 512], F32, tag="rbs")
              nc.gpsimd.partition_broadcast(
                  rbs[:], recrow[h][:, 4 * gq:4 * gq + 4, :])
              ct = avps.tile([128, 512], F32, tag="ct")
              last = 4 * gq + 3
              for j in range(last + 1):
                  if j <= 4 * gq:
                      nc.tensor.matmul(ct[:], vres[j][:, h * 128:(h + 1) * 128],
                                       expT[h][j][:, (4 * gq - j) * 128:(4 * gq - j) * 128 + 512],
                                       start=(j == 0), stop=(j == last))
                  else:
                      w = (4 * gq + 4 - j) * 128
                      nc.tensor.matmul(ct[:, 512 - w:512], vres[j][:, h * 128:(h + 1) * 128],
                                       expT[h][j][:, 0:w], start=False, stop=(j == last))
              nc.vector.tensor_tensor(out=ct_sb[(h, gq)][:], in0=ct[:], in1=rbs[:], op=MULT)

          lrec_cur = {}
          for h in range(HG):
              expT[h] = []
              lrec_cur[h] = lrp.tile([128, 16], F32, tag="lrec", name="lrec")
              recrow[h] = lrp.tile([1, 16, 128], F32, tag="recrow", name="recrow")
              for i in range(NB):
                  w = (NB - i) * 128
                  ex = expp.tile([128, w], BF16, tag=f"expT{i}", name=f"expT{i}")
                  expT[h].append(ex)
                  for c0 in range(0, w, 1024):
                      cw = min(1024, w - c0)
                      sp = sps.tile([128, 1024], F32, tag="sp")
                      for s5 in range(0, cw, 512):
                          w5 = min(512, cw - s5)
                          q0 = i * 128 + c0 + s5
                          nc.tensor.matmul(sp[:, s5:s5 + w5],
                                           qt[HG + h][:, i * 128:(i + 1) * 128],
                                           qt[h][:, q0:q0 + w5], start=True, stop=True)
                      if c0 == 0:
                          nc.vector.tensor_tensor(out=sp[:, 0:128], in0=sp[:, 0:128],
                                                  in1=tmask[:], op=ADD)
                      nc.scalar.activation(out=ex[:, c0:c0 + cw], in_=sp[:, 0:cw],
                                           func=mybir.ActivationFunctionType.Exp,
                                           scale=SCALE)
                  if i >= 2:
                      denom(h, i - 2)
                  if h >= 1 and i % 4 == 3:
                      sweep2_group(h - 1, i // 4)
              denom(h, NB - 2)
              denom(h, NB - 1)
              # recip row: [128,16] -> transpose -> [16,128] -> DRAM -> [1,2048]
              rt = smps.tile([16, 128], F32, tag="rt")
              nc.tensor.transpose(rt[:], lrec_cur[h][:], ident_f[:])
              rts = lrp.tile([16, 128], F32, tag="rts")
              nc.vector.tensor_copy(out=rts[:], in_=rt[:])
              nc.sync.dma_start(out=lrt[h, 0], in_=rts[:])
              nc.sync.dma_start(out=recrow[h][:], in_=lrt[h])

          # close S^T/denom psum pools before opening phase-3 psum
          p2inner.close()

          # ---------------- Phase 3: output projection, interleaving head-3 sweep2 ----
          with tc.tile_pool(name="p3s", bufs=4) as p3s, \
               tc.tile_pool(name="p3ps", bufs=kn["p3ps"], space="PSUM") as p3ps:
              for gq in range(4):
                  sweep2_group(HG - 1, gq)
                  for mt in range(16):
                      op = p3ps.tile([128, 512], F32, tag="op")
                      for kh in range(HG):
                          nc.tensor.matmul(op[:], two[:, kh, mt * 128:(mt + 1) * 128],
                                           ct_sb[(kh, gq)][:],
                                           start=(kh == 0), stop=(kh == 3))
                      ob = p3s.tile([128, 512], BF16, tag="ob")
                      if mt % 2 == 0:
                          nc.vector.tensor_copy(out=ob[:], in_=op[:])
                      else:
                          nc.scalar.copy(out=ob[:], in_=op[:])
                      nc.sync.dma_start(out=outt[mt, :, gq * 512:(gq + 1) * 512], in_=ob[:])
          p2stack.close()
    nc.finalize()
    return nc


_NC_CACHE = {}


def _get_nc(reps=1):
    if reps not in _NC_CACHE:
        _NC_CACHE[reps] = build_nc(reps)
    return _NC_CACHE[reps]


def _rope_tables(position_ids_b):
    pos = position_ids_b.astype(np.float32)
    inv_freq = (1.0 / (ROPE_THETA ** (np.arange(0, DH, 2, dtype=np.float32) / np.float32(DH))))
    ang = pos[:, None] * inv_freq[None, :]          # [S, 64]
    emb = np.concatenate([ang, ang], axis=-1)       # [S, 128]
    cosT = np.ascontiguousarray(np.cos(emb).T)      # [128, S]
    sinT = np.sin(emb).T
    sin_rot = np.concatenate([-sinT[0:64], sinT[64:128]], axis=0)
    return cosT.astype(ml_dtypes.bfloat16), np.ascontiguousarray(sin_rot).astype(ml_dtypes.bfloat16)


def _make_in_maps(inputs):
    hidden_states = np.asarray(inputs["hidden_states"], dtype=np.float32)
    position_ids = np.asarray(inputs["position_ids"])
    Wqkv = np.asarray(inputs["Wqkv"], dtype=np.float32)
    bqkv = np.asarray(inputs["bqkv"], dtype=np.float32)
    Wo = np.asarray(inputs["Wo"], dtype=np.float32)

    mask = np.tril(np.full((128, 128), NEG, dtype=np.float32), k=-1)
    tabs = [_rope_tables(np.asarray(position_ids)[b]) for b in range(B)]

    def _hilo(M, sc):
        Ms = M * np.float32(sc)
        hi = Ms.astype(ml_dtypes.float8_e4m3)
        lo = (Ms - hi.astype(np.float32)).astype(ml_dtypes.float8_e4m3)
        return hi, lo

    def _pack_pairs(M):
        # [D, C] -> [8, 128, 2, C] with row r = kc2*256 + i*128 + p
        C = M.shape[1]
        return np.ascontiguousarray(M.reshape(8, 2, 128, C).transpose(0, 2, 1, 3))

    def _pack_x(M):
        # [D, S] -> [128, 8, 2, S] partition-major
        C = M.shape[1]
        return np.ascontiguousarray(M.reshape(8, 2, 128, C).transpose(2, 0, 1, 3))

    xts = []
    for b in range(B):
        hi, lo = _hilo(np.ascontiguousarray(hidden_states[b].T), XSC)
        xts.append((_pack_x(hi.astype(np.float32)).astype(ml_dtypes.float8_e4m3),
                    _pack_x(lo.astype(np.float32)).astype(ml_dtypes.float8_e4m3)))
    onesb = np.ones((128, 1), dtype=ml_dtypes.bfloat16)

    in_maps = []
    for c in range(NCORES):
        b, hg = divmod(c, HG)
        qcols = slice(hg * GQ, (hg + 1) * GQ)
        kcols = slice(D + hg * GQ, D + (hg + 1) * GQ)
        vcols = slice(2 * D + hg * GQ, 2 * D + (hg + 1) * GQ)
        wqk_c = np.ascontiguousarray(np.concatenate([Wqkv[:, qcols], Wqkv[:, kcols]], axis=1))
        qk_h, qk_l = _hilo(wqk_c, WSC)
        # per-mt packing: [8(mt), 128(p), 8(kc2), 2(i), 128(m)]
        def _pack_mt(M8):
            P = _pack_pairs(M8.astype(np.float32))          # [8, 128, 2, 1024]
            P = P.reshape(8, 128, 2, 8, 128)                 # [kc2, p, i, mt, m]
            return np.ascontiguousarray(P.transpose(3, 1, 0, 2, 4)).astype(ml_dtypes.float8_e4m3)
        wqkh_c = _pack_mt(qk_h)
        wqkl_c = _pack_mt(qk_l)
        wv_c = np.ascontiguousarray(Wqkv[:, vcols])
        v_h, v_l = _hilo(wv_c, WSC)
        wvh_c = _pack_pairs(v_h.astype(np.float32)).astype(ml_dtypes.float8_e4m3)
        wvl_c = _pack_pairs(v_l.astype(np.float32)).astype(ml_dtypes.float8_e4m3)
        wo_c = np.ascontiguousarray(Wo[hg * GQ:(hg + 1) * GQ, :]).astype(ml_dtypes.bfloat16).reshape(4, 128, D)
        bqk_c = np.concatenate([bqkv[qcols], bqkv[kcols]]).reshape(8, 128).T
        bv_c = bqkv[vcols].reshape(1, GQ)
        cosT, sin_rot = tabs[b]
        in_maps.append({
            "xh": xts[b][0], "xl": xts[b][1],
            "wqkh": wqkh_c, "wqkl": wqkl_c, "wvh": wvh_c, "wvl": wvl_c, "wo": wo_c,
            "bqkt": np.ascontiguousarray(bqk_c),
            "bv": np.ascontiguousarray(bv_c),
            "cost": cosT, "sinrt": sin_rot, "maskd": mask,
            "onesb": onesb,
        })
    return in_maps


def kernel(hidden_states, position_ids, Wqkv, bqkv, Wo, bo, _reps=1):
    bo = np.asarray(bo, dtype=np.float32)
    in_maps = _make_in_maps({
        "hidden_states": hidden_states, "position_ids": position_ids,
        "Wqkv": Wqkv, "bqkv": bqkv, "Wo": Wo, "bo": bo,
    })
    nc = _get_nc(_reps)
    res = run_bass_kernel_spmd(nc, in_maps, core_ids=list(range(NCORES)))

    out = np.empty((B, S, D), dtype=np.float32)
    for b in range(B):
        acc = res.results[b * HG]["outt"].reshape(D, S).astype(np.float32).copy()
        for hg in range(1, HG):
            acc += res.results[b * HG + hg]["outt"].reshape(D, S).astype(np.float32)
        out[b] = acc.T + bo[None, :]
    return out


# revision 16
# speedup vs baseline: 1.3133x; 1.0500x over previous
"""Trainium2 Bass kernel for CustomRoPEAttention (B=2, S=2048, H=16, Dh=128).

Sharding: 8 cores = 2 batches x 4 head-groups (4 heads/core). Each core:
QKV projection (bf16 matmuls) + RoPE for its heads, transposed-layout causal
attention (scores computed as S^T with keys on partitions so the attention
probabilities feed A@V directly as the moving operand -- no PE transposes),
softmax denominators via ap-size-1 matmuls + deferred normalization, and a
partial (transposed) output projection. Host sums the 4 partials per batch.

Self-contained: hardcodes shapes from the problem spec.
"""
import math
from contextlib import ExitStack

import numpy as np
import ml_dtypes

import concourse.mybir as mybir
import concourse.tile as tile
from concourse import bacc
from concourse.bass_utils import run_bass_kernel_spmd
from concourse.masks import make_identity

S = 2048            # sequence
D = 2048            # hidden
NH = 16             # total heads
DH = 128            # head dim
HG = 4              # heads per core
GQ = HG * DH        # 512: per-core q/k/v feature width
B = 2
NCORES = 8
ROPE_THETA = 10000.0
SCALE = 1.0 / math.sqrt(DH)
NEG = -1.0e9
SLAB = 512          # phase-1 sequence slab width
XSC = 16.0          # fp8 pre-scale for x (keeps hi/lo in e4m3 normal range)
WSC = 512.0         # fp8 pre-scale for Wqkv
INV_SC = 1.0 / (XSC * WSC)
F32 = mybir.dt.float32
BF16 = mybir.dt.bfloat16
F16 = mybir.dt.float16
MULT = mybir.AluOpType.mult
ADD = mybir.AluOpType.add
NB = S // 128       # 16 k/q blocks


def build_nc(reps=1, knobs=None):
    kn = {"p1ps": 3, "p1vps": 2, "p1x": 2, "sps": 3, "avps": 1, "p3ps": 4}
    if knobs:
        kn.update(knobs)
    nc = bacc.Bacc(None, target_bir_lowering=False)
    F8 = mybir.dt.float8e4
    xh = nc.dram_tensor("xh", [128, 8, 2, S], F8, kind="ExternalInput")
    xl = nc.dram_tensor("xl", [128, 8, 2, S], F8, kind="ExternalInput")
    # per-mt packed qk weights: [mt, p, kc2, i, m]
    wqkh = nc.dram_tensor("wqkh", [8, 128, 8, 2, 128], F8, kind="ExternalInput")
    wqkl = nc.dram_tensor("wqkl", [8, 128, 8, 2, 128], F8, kind="ExternalInput")
    wvh = nc.dram_tensor("wvh", [8, 128, 2, GQ], F8, kind="ExternalInput")
    wvl = nc.dram_tensor("wvl", [8, 128, 2, GQ], F8, kind="ExternalInput")
    wo = nc.dram_tensor("wo", [4, 128, D], BF16, kind="ExternalInput")
    bqkt = nc.dram_tensor("bqkt", [128, 8], F32, kind="ExternalInput")
    bv = nc.dram_tensor("bv", [1, GQ], F32, kind="ExternalInput")
    cost = nc.dram_tensor("cost", [128, S], BF16, kind="ExternalInput")    # cos^T
    sinrt = nc.dram_tensor("sinrt", [128, S], BF16, kind="ExternalInput")  # sin^T, rot sign
    maskd = nc.dram_tensor("maskd", [128, 128], F32, kind="ExternalInput")  # tril(-1) NEG
    onesb = nc.dram_tensor("onesb", [128, 1], BF16, kind="ExternalInput")
    outt = nc.dram_tensor("outt", [16, 128, S], BF16, kind="ExternalOutput")
    lrt = nc.dram_tensor("lrt", [HG, 1, 16, 128], F32)  # recip bounce: [16,128] -> [1,2048]

    with tile.TileContext(nc) as tc, ExitStack() as top:
        g = top.enter_context(tc.tile_pool(name="glob", bufs=1))
        tcos = g.tile([128, S], BF16)
        tsin = g.tile([128, S], BF16)
        tmask = g.tile([128, 128], F32)
        ident_f = g.tile([128, 128], F32)
        make_identity(nc, ident_f[:])
        tbqkt = g.tile([128, 8], F32)
        nc.sync.dma_start(out=tbqkt, in_=bqkt[:])
        tbvb = g.tile([128, GQ], F32)
        tones = g.tile([128, 1], BF16)
        tinv = g.tile([128, 1], F32)
        nc.vector.memset(tinv[:], INV_SC)

        def load_consts():
            nc.sync.dma_start(out=tcos, in_=cost[:])
            nc.sync.dma_start(out=tsin, in_=sinrt[:])
            nc.sync.dma_start(out=tmask, in_=maskd[:])
            nc.sync.dma_start(out=tbvb, in_=bv[:].to_broadcast((128, GQ)))
            nc.sync.dma_start(out=tones, in_=onesb[:])

        # Whole-kernel residents
        res = top.enter_context(tc.tile_pool(name="res", bufs=1))
        qt = []  # mt 0..3 = Q^T heads, 4..7 = K^T heads, each [128(dh), S] bf16
        for mt in range(2 * HG):
            qt.append(res.tile([128, S], BF16, tag=f"qt{mt}", name=f"qt{mt}"))
        vres = []  # 16 V k-block tiles [128(seq), GQ] bf16
        for t in range(NB):
            vres.append(res.tile([128, GQ], BF16, tag=f"v{t}", name=f"v{t}"))
        two = res.tile([128, 4, D], BF16, tag="two")
        ct_sb = {}
        for h in range(HG):
            for gq in range(4):
                ct_sb[(h, gq)] = res.tile([128, 512], BF16, tag=f"ct_{h}_{gq}", name=f"ct_{h}_{gq}")

        for _rep in range(reps):
          # ---------------- Phase 1: QKV^T projection + RoPE (all resident) ----------
          with tc.tile_pool(name="p1w", bufs=1) as p1w, \
               tc.tile_pool(name="p1x", bufs=kn["p1x"]) as p1x, \
               tc.tile_pool(name="qswp", bufs=2) as qswp, \
               tc.tile_pool(name="p1stg", bufs=1) as p1stg, \
               tc.tile_pool(name="p1ps", bufs=kn["p1ps"], space="PSUM") as p1ps, \
               tc.tile_pool(name="p1vps", bufs=kn["p1vps"], space="PSUM") as p1vps:
            # startup-critical order: xs0h per-kc2, per-mt wqk stream (h then l),
            # xs0l, xs1, wv, consts
            F8 = mybir.dt.float8e4
            DR = mybir.MatmulPerfMode.DoubleRow
            twqkh, twqkl, twvh, twvl = [], [], [], []
            wt = p1w.tile([128, 8, 2, 128], F8, tag="wqkh0", name="wqkh0")
            nc.sync.dma_start(out=wt, in_=wqkh[0])
            twqkh.append(wt)
            xs0h = p1x.tile([128, 8, 2, SLAB], F8, tag="xsh", name="xs0h")
            for kc2 in range(8):
                nc.sync.dma_start(out=xs0h[:, kc2, :, :], in_=xh[:, kc2, :, 0:SLAB])
            wt = p1w.tile([128, 8, 2, 128], F8, tag="wqkl0", name="wqkl0")
            nc.sync.dma_start(out=wt, in_=wqkl[0])
            twqkl.append(wt)
            xs0l = p1x.tile([128, 8, 2, SLAB], F8, tag="xsl", name="xs0l")
            for kc2 in range(8):
                nc.sync.dma_start(out=xs0l[:, kc2, :, :], in_=xl[:, kc2, :, 0:SLAB])
            for mt in range(1, 8):
                wt = p1w.tile([128, 8, 2, 128], F8, tag=f"wqkh{mt}")
                nc.sync.dma_start(out=wt, in_=wqkh[mt])
                twqkh.append(wt)
                wt = p1w.tile([128, 8, 2, 128], F8, tag=f"wqkl{mt}")
                nc.sync.dma_start(out=wt, in_=wqkl[mt])
                twqkl.append(wt)
            xs1h = p1x.tile([128, 8, 2, SLAB], F8, tag="xsh", name="xs1h")
            nc.sync.dma_start(out=xs1h, in_=xh[:, :, :, SLAB:2 * SLAB])
            xs1l = p1x.tile([128, 8, 2, SLAB], F8, tag="xsl", name="xs1l")
            nc.sync.dma_start(out=xs1l, in_=xl[:, :, :, SLAB:2 * SLAB])
            for kc2 in range(8):
                wt = p1w.tile([128, 2, GQ], F8, tag=f"wvh{kc2}")
                nc.sync.dma_start(out=wt, in_=wvh[kc2])
                twvh.append(wt)
                wt = p1w.tile([128, 2, GQ], F8, tag=f"wvl{kc2}")
                nc.sync.dma_start(out=wt, in_=wvl[kc2])
                twvl.append(wt)
            load_consts()
            xs_pre = {0: (xs0h, xs0l), 1: (xs1h, xs1l)}
            for ns in range(S // SLAB):
                sl = slice(ns * SLAB, (ns + 1) * SLAB)
                if ns in xs_pre:
                    xsh, xsl = xs_pre[ns]
                else:
                    xsh = p1x.tile([128, 8, 2, SLAB], F8, tag="xsh", name=f"xs{ns}h")
                    nc.sync.dma_start(out=xsh, in_=xh[:, :, :, sl])
                    xsl = p1x.tile([128, 8, 2, SLAB], F8, tag="xsl", name=f"xs{ns}l")
                    nc.sync.dma_start(out=xsl, in_=xl[:, :, :, sl])
                for mt in range(2 * HG):
                    pqk = p1ps.tile([128, SLAB], F32, tag="qkps")
                    passes = [(twqkh[mt], xsh), (twqkh[mt], xsl), (twqkl[mt], xsh)]
                    np_ = len(passes)
                    for pi, (wt, xt_) in enumerate(passes):
                        for kc2 in range(8):
                            nc.tensor.matmul(pqk[:], wt[:, kc2, :, :], xt_[:, kc2, :, :],
                                             start=(pi == 0 and kc2 == 0),
                                             stop=(pi == np_ - 1 and kc2 == 7),
                                             perf_mode=DR)
                    nc.scalar.activation(out=qt[mt][:, sl], in_=pqk[:],
                                         func=mybir.ActivationFunctionType.Identity,
                                         scale=INV_SC, bias=tbqkt[:, mt:mt + 1])
                for st in range(SLAB // 128):
                    pv = p1vps.tile([128, GQ], F32, tag="vps")
                    s0 = st * 128
                    passes = [(xsh, twvh), (xsl, twvh), (xsh, twvl)]
                    np_ = len(passes)
                    for pi, (xt_, wv_) in enumerate(passes):
                        for kc2 in range(8):
                            nc.tensor.matmul(pv[:], xt_[:, kc2, :, s0:s0 + 128],
                                             wv_[kc2][:], start=(pi == 0 and kc2 == 0),
                                             stop=(pi == np_ - 1 and kc2 == 7),
                                             perf_mode=DR)
                    nc.vector.scalar_tensor_tensor(
                        out=vres[ns * (SLAB // 128) + st], in0=pv[:],
                        scalar=tinv[:], in1=tbvb[:], op0=MULT, op1=ADD)
            # RoPE per head tile (full width): q' = qb*cos + swap(qb)*sin_rot
            # (bias already applied in the psum copy); head 0's q,k first
            for mt in (0, 4, 1, 5, 2, 6, 3, 7):
                qsw = qswp.tile([128, S], BF16, tag="qsw")
                nc.sync.dma_start(out=qsw[0:64, :], in_=qt[mt][64:128, :])
                nc.sync.dma_start(out=qsw[64:128, :], in_=qt[mt][0:64, :])
                m1 = p1stg.tile([128, S], BF16, tag="m1")
                nc.vector.tensor_tensor(out=m1[:], in0=qt[mt][:], in1=tcos[:], op=MULT)
                m2 = p1stg.tile([128, S], BF16, tag="m2")
                nc.vector.tensor_tensor(out=m2[:], in0=qsw[:], in1=tsin[:], op=MULT)
                nc.vector.tensor_tensor(out=qt[mt][:], in0=m1[:], in1=m2[:], op=ADD)

          # ---------------- Phase 2: transposed attention ----------------
          p2stack = ExitStack()
          expp = p2stack.enter_context(tc.tile_pool(name="expp", bufs=2))
          lrp = p2stack.enter_context(tc.tile_pool(name="lrp", bufs=2))
          avps = p2stack.enter_context(
              tc.tile_pool(name="avps", bufs=kn["avps"], space="PSUM"))
          rbp = p2stack.enter_context(tc.tile_pool(name="rbp", bufs=2))
          p2inner = ExitStack()
          sps = p2inner.enter_context(
              tc.tile_pool(name="sps", bufs=kn["sps"], space="PSUM"))
          smps = p2inner.enter_context(tc.tile_pool(name="smps", bufs=1, space="PSUM"))
          smt = smps.tile([128, 132], F32, tag="sm", name="smt")

          nc.sync.dma_start(out=two, in_=wo.rearrange("kc p f -> p kc f"))
          expT = [None] * HG  # per live head: list of 16 exp(S^T) tiles
          recrow = [None] * HG

          def denom(h, b):
              # ell[q] for q-block b: sum_k exp tiles, ap-1 matmuls, then recip
              lp = smt[:, 128 + (b % 4):129 + (b % 4)]
              for j in range(b + 1):
                  nc.tensor.matmul(lp, expT[h][j][:, (b - j) * 128:(b - j + 1) * 128],
                                   tones[:], start=(j == 0), stop=(j == b))
              nc.vector.reciprocal(out=lrec_cur[h][:, b:b + 1], in_=lp)

          def sweep2_group(h, gq):
              # normalize+accumulate: ct = (sum_k V^T[k] expS^T[k]) * recip(ell)
              rbs = rbp.tile([128, 512], F32, tag="rbs")
              nc.gpsimd.partition_broadcast(
                  rbs[:], recrow[h][:, 4 * gq:4 * gq + 4, :])
              ct = avps.tile([128, 512], F32, tag="ct")
              last = 4 * gq + 3
              for j in range(last + 1):
                  if j <= 4 * gq:
                      nc.tensor.matmul(ct[:], vres[j][:, h * 128:(h + 1) * 128],
                                       expT[h][j][:, (4 * gq - j) * 128:(4 * gq - j) * 128 + 512],
                                       start=(j == 0), stop=(j == last))
                  else:
                      w = (4 * gq + 4 - j) * 128
                      nc.tensor.matmul(ct[:, 512 - w:512], vres[j][:, h * 128:(h + 1) * 128],
                                       expT[h][j][:, 0:w], start=False, stop=(j == last))
              nc.vector.tensor_tensor(out=ct_sb[(h, gq)][:], in0=ct[:], in1=rbs[:], op=MULT)

          lrec_cur = {}
          for h in range(HG):
              expT[h] = []
              lrec_cur[h] = lrp.tile([128, 16], F32, tag="lrec", name="lrec")
              recrow[h] = lrp.tile([1, 16, 128], F32, tag="recrow", name="recrow")
              for i in range(NB):
                  w = (NB - i) * 128
                  ex = expp.tile([128, w], BF16, tag=f"expT{i}", name=f"expT{i}")
                  expT[h].append(ex)
                  for c0 in range(0, w, 1024):
                      cw = min(1024, w - c0)
                      sp = sps.tile([128, 1024], F32, tag="sp")
                      for s5 in range(0, cw, 512):
                          w5 = min(512, cw - s5)
                          q0 = i * 128 + c0 + s5
                          nc.tensor.matmul(sp[:, s5:s5 + w5],
                                           qt[HG + h][:, i * 128:(i + 1) * 128],
                                           qt[h][:, q0:q0 + w5], start=True, stop=True)
                      if c0 == 0:
                          nc.vector.tensor_tensor(out=sp[:, 0:128], in0=sp[:, 0:128],
                                                  in1=tmask[:], op=ADD)
                      nc.scalar.activation(out=ex[:, c0:c0 + cw], in_=sp[:, 0:cw],
                                           func=mybir.ActivationFunctionType.Exp,
                                           scale=SCALE)
                  if i >= 2:
                      denom(h, i - 2)
                  if h >= 1 and i % 4 == 3:
                      sweep2_group(h - 1, i // 4)
              denom(h, NB - 2)
              denom(h, NB - 1)
              # recip row: [128,16] -> transpose -> [16,128] -> DRAM -> [1,2048]
              rt = smt[0:16, 0:128]
              nc.tensor.transpose(rt, lrec_cur[h][:], ident_f[:])
              rts = lrp.tile([16, 128], F32, tag="rts")
              nc.vector.tensor_copy(out=rts[:], in_=rt)
              nc.sync.dma_start(out=lrt[h, 0], in_=rts[:])
              nc.sync.dma_start(out=recrow[h][:], in_=lrt[h])

          # close S^T/denom psum pools before opening phase-3 psum
          p2inner.close()

          # ---------------- Phase 3: output projection, interleaving head-3 sweep2 ----
          with tc.tile_pool(name="p3s", bufs=4) as p3s, \
               tc.tile_pool(name="p3ps", bufs=kn["p3ps"], space="PSUM") as p3ps:
              sweep2_group(HG - 1, 0)
              for gq in range(4):
                  if gq + 1 < 4:
                      sweep2_group(HG - 1, gq + 1)
                  for mt in range(16):
                      op = p3ps.tile([128, 512], F32, tag="op")
                      for kh in range(HG):
                          nc.tensor.matmul(op[:], two[:, kh, mt * 128:(mt + 1) * 128],
                                           ct_sb[(kh, gq)][:],
                                           start=(kh == 0), stop=(kh == 3))
                      ob = p3s.tile([128, 512], BF16, tag="ob")
                      if mt % 2 == 0:
                          nc.vector.tensor_copy(out=ob[:], in_=op[:])
                      else:
                          nc.scalar.copy(out=ob[:], in_=op[:])
                      nc.sync.dma_start(out=outt[mt, :, gq * 512:(gq + 1) * 512], in_=ob[:])
          p2stack.close()
    nc.finalize()
    return nc


_NC_CACHE = {}


def _get_nc(reps=1):
    if reps not in _NC_CACHE:
        _NC_CACHE[reps] = build_nc(reps)
    return _NC_CACHE[reps]


def _rope_tables(position_ids_b):
    pos = position_ids_b.astype(np.float32)
    inv_freq = (1.0 / (ROPE_THETA ** (np.arange(0, DH, 2, dtype=np.float32) / np.float32(DH))))
    ang = pos[:, None] * inv_freq[None, :]          # [S, 64]
    emb = np.concatenate([ang, ang], axis=-1)       # [S, 128]
    cosT = np.ascontiguousarray(np.cos(emb).T)      # [128, S]
    sinT = np.sin(emb).T
    sin_rot = np.concatenate([-sinT[0:64], sinT[64:128]], axis=0)
    return cosT.astype(ml_dtypes.bfloat16), np.ascontiguousarray(sin_rot).astype(ml_dtypes.bfloat16)


def _make_in_maps(inputs):
    hidden_states = np.asarray(inputs["hidden_states"], dtype=np.float32)
    position_ids = np.asarray(inputs["position_ids"])
    Wqkv = np.asarray(inputs["Wqkv"], dtype=np.float32)
    bqkv = np.asarray(inputs["bqkv"], dtype=np.float32)
    Wo = np.asarray(inputs["Wo"], dtype=np.float32)

    mask = np.tril(np.full((128, 128), NEG, dtype=np.float32), k=-1)
    tabs = [_rope_tables(np.asarray(position_ids)[b]) for b in range(B)]

    def _hilo(M, sc):
        Ms = M * np.float32(sc)
        hi = Ms.astype(ml_dtypes.float8_e4m3)
        lo = (Ms - hi.astype(np.float32)).astype(ml_dtypes.float8_e4m3)
        return hi, lo

    def _pack_pairs(M):
        # [D, C] -> [8, 128, 2, C] with row r = kc2*256 + i*128 + p
        C = M.shape[1]
        return np.ascontiguousarray(M.reshape(8, 2, 128, C).transpose(0, 2, 1, 3))

    def _pack_x(M):
        # [D, S] -> [128, 8, 2, S] partition-major
        C = M.shape[1]
        return np.ascontiguousarray(M.reshape(8, 2, 128, C).transpose(2, 0, 1, 3))

    xts = []
    for b in range(B):
        hi, lo = _hilo(np.ascontiguousarray(hidden_states[b].T), XSC)
        xts.append((_pack_x(hi.astype(np.float32)).astype(ml_dtypes.float8_e4m3),
                    _pack_x(lo.astype(np.float32)).astype(ml_dtypes.float8_e4m3)))
    onesb = np.ones((128, 1), dtype=ml_dtypes.bfloat16)

    in_maps = []
    for c in range(NCORES):
        b, hg = divmod(c, HG)
        qcols = slice(hg * GQ, (hg + 1) * GQ)
        kcols = slice(D + hg * GQ, D + (hg + 1) * GQ)
        vcols = slice(2 * D + hg * GQ, 2 * D + (hg + 1) * GQ)
        wqk_c = np.ascontiguousarray(np.concatenate([Wqkv[:, qcols], Wqkv[:, kcols]], axis=1))
        qk_h, qk_l = _hilo(wqk_c, WSC)
        # per-mt packing: [8(mt), 128(p), 8(kc2), 2(i), 128(m)]
        def _pack_mt(M8):
            P = _pack_pairs(M8.astype(np.float32))          # [8, 128, 2, 1024]
            P = P.reshape(8, 128, 2, 8, 128)                 # [kc2, p, i, mt, m]
            return np.ascontiguousarray(P.transpose(3, 1, 0, 2, 4)).astype(ml_dtypes.float8_e4m3)
        wqkh_c = _pack_mt(qk_h)
        wqkl_c = _pack_mt(qk_l)
        wv_c = np.ascontiguousarray(Wqkv[:, vcols])
        v_h, v_l = _hilo(wv_c, WSC)
        wvh_c = _pack_pairs(v_h.astype(np.float32)).astype(ml_dtypes.float8_e4m3)
        wvl_c = _pack_pairs(v_l.astype(np.float32)).astype(ml_dtypes.float8_e4m3)
        wo_c = np.ascontiguousarray(Wo[hg * GQ:(hg + 1) * GQ, :]).astype(ml_dtypes.bfloat16).reshape(4, 128, D)
        bqk_c = np.concatenate([bqkv[qcols], bqkv[kcols]]).reshape(8, 128).T
        bv_c = bqkv[vcols].reshape(1, GQ)
        cosT, sin_rot = tabs[b]
        in_maps.append({
            "xh": xts[b][0], "xl": xts[b][1],
            "wqkh": wqkh_c, "wqkl": wqkl_c, "wvh": wvh_c, "wvl": wvl_c, "wo": wo_c,
            "bqkt": np.ascontiguousarray(bqk_c),
            "bv": np.ascontiguousarray(bv_c),
            "cost": cosT, "sinrt": sin_rot, "maskd": mask,
            "onesb": onesb,
        })
    return in_maps


def kernel(hidden_states, position_ids, Wqkv, bqkv, Wo, bo, _reps=1):
    bo = np.asarray(bo, dtype=np.float32)
    in_maps = _make_in_maps({
        "hidden_states": hidden_states, "position_ids": position_ids,
        "Wqkv": Wqkv, "bqkv": bqkv, "Wo": Wo, "bo": bo,
    })
    nc = _get_nc(_reps)
    res = run_bass_kernel_spmd(nc, in_maps, core_ids=list(range(NCORES)))

    out = np.empty((B, S, D), dtype=np.float32)
    for b in range(B):
        acc = res.results[b * HG]["outt"].reshape(D, S).astype(np.float32).copy()
        for hg in range(1, HG):
            acc += res.results[b * HG + hg]["outt"].reshape(D, S).astype(np.float32)
        out[b] = acc.T + bo[None, :]
    return out


# revision 17
# speedup vs baseline: 1.3742x; 1.0464x over previous
"""Trainium2 Bass kernel for CustomRoPEAttention (B=2, S=2048, H=16, Dh=128).

Sharding: 8 cores = 2 batches x 4 head-groups (4 heads/core). Each core:
QKV projection (bf16 matmuls) + RoPE for its heads, transposed-layout causal
attention (scores computed as S^T with keys on partitions so the attention
probabilities feed A@V directly as the moving operand -- no PE transposes),
softmax denominators via ap-size-1 matmuls + deferred normalization, and a
partial (transposed) output projection. Host sums the 4 partials per batch.

Self-contained: hardcodes shapes from the problem spec.
"""
import math
from contextlib import ExitStack

import numpy as np
import ml_dtypes

import concourse.mybir as mybir
import concourse.tile as tile
from concourse import bacc
from concourse.bass_utils import run_bass_kernel_spmd
from concourse.masks import make_identity

S = 2048            # sequence
D = 2048            # hidden
NH = 16             # total heads
DH = 128            # head dim
HG = 4              # heads per core
GQ = HG * DH        # 512: per-core q/k/v feature width
B = 2
NCORES = 8
ROPE_THETA = 10000.0
SCALE = 1.0 / math.sqrt(DH)
NEG = -1.0e9
SLAB = 512          # phase-1 sequence slab width
XSC = 16.0          # fp8 pre-scale for x (keeps hi/lo in e4m3 normal range)
WSC = 512.0         # fp8 pre-scale for Wqkv
INV_SC = 1.0 / (XSC * WSC)
F32 = mybir.dt.float32
BF16 = mybir.dt.bfloat16
F16 = mybir.dt.float16
MULT = mybir.AluOpType.mult
ADD = mybir.AluOpType.add
NB = S // 128       # 16 k/q blocks


def build_nc(reps=1, knobs=None):
    kn = {"p1ps": 3, "p1vps": 2, "p1x": 2, "sps": 3, "avps": 1, "p3ps": 4}
    if knobs:
        kn.update(knobs)
    nc = bacc.Bacc(None, target_bir_lowering=False)
    F8 = mybir.dt.float8e4
    xh = nc.dram_tensor("xh", [128, 8, 2, S], F8, kind="ExternalInput")
    xl = nc.dram_tensor("xl", [128, 8, 2, S], F8, kind="ExternalInput")
    # per-mt packed qk weights: [mt, p, kc2, i, m]
    wqkh = nc.dram_tensor("wqkh", [8, 128, 8, 2, 128], F8, kind="ExternalInput")
    wqkl = nc.dram_tensor("wqkl", [8, 128, 8, 2, 128], F8, kind="ExternalInput")
    wvh = nc.dram_tensor("wvh", [8, 128, 2, GQ], F8, kind="ExternalInput")
    wvl = nc.dram_tensor("wvl", [8, 128, 2, GQ], F8, kind="ExternalInput")
    wo = nc.dram_tensor("wo", [4, 128, D], BF16, kind="ExternalInput")
    bqkt = nc.dram_tensor("bqkt", [128, 8], F32, kind="ExternalInput")
    bv = nc.dram_tensor("bv", [1, GQ], F32, kind="ExternalInput")
    cost = nc.dram_tensor("cost", [128, S], BF16, kind="ExternalInput")    # cos^T
    sinrt = nc.dram_tensor("sinrt", [128, S], BF16, kind="ExternalInput")  # sin^T, rot sign
    maskd = nc.dram_tensor("maskd", [128, 128], F32, kind="ExternalInput")  # tril(-1) NEG
    onesb = nc.dram_tensor("onesb", [128, 1], BF16, kind="ExternalInput")
    outt = nc.dram_tensor("outt", [16, 128, S], BF16, kind="ExternalOutput")
    lrt = nc.dram_tensor("lrt", [HG, 1, 16, 128], F32)  # recip bounce: [16,128] -> [1,2048]

    with tile.TileContext(nc) as tc, ExitStack() as top:
        g = top.enter_context(tc.tile_pool(name="glob", bufs=1))
        tcos = g.tile([128, S], BF16)
        tsin = g.tile([128, S], BF16)
        tmask = g.tile([128, 128], F32)
        ident_f = g.tile([128, 128], F32)
        make_identity(nc, ident_f[:])
        tbqkt = g.tile([128, 8], F32)
        nc.sync.dma_start(out=tbqkt, in_=bqkt[:])
        tbvb = g.tile([128, GQ], F32)
        tones = g.tile([128, 1], BF16)
        tinv = g.tile([128, 1], F32)
        nc.vector.memset(tinv[:], INV_SC)

        def load_consts():
            nc.sync.dma_start(out=tcos, in_=cost[:])
            nc.sync.dma_start(out=tsin, in_=sinrt[:])
            nc.sync.dma_start(out=tmask, in_=maskd[:])
            nc.sync.dma_start(out=tbvb, in_=bv[:].to_broadcast((128, GQ)))
            nc.sync.dma_start(out=tones, in_=onesb[:])

        # Whole-kernel residents
        res = top.enter_context(tc.tile_pool(name="res", bufs=1))
        qt = []  # mt 0..3 = Q^T heads, 4..7 = K^T heads, each [128(dh), S] bf16
        for mt in range(2 * HG):
            qt.append(res.tile([128, S], BF16, tag=f"qt{mt}", name=f"qt{mt}"))
        vres = []  # 16 V k-block tiles [128(seq), GQ] bf16
        for t in range(NB):
            vres.append(res.tile([128, GQ], BF16, tag=f"v{t}", name=f"v{t}"))
        two = res.tile([128, 4, D], BF16, tag="two")
        ct_sb = {}
        for h in range(HG):
            for gq in range(4):
                ct_sb[(h, gq)] = res.tile([128, 512], BF16, tag=f"ct_{h}_{gq}", name=f"ct_{h}_{gq}")

        for _rep in range(reps):
          # ---------------- Phase 1: QKV^T projection + RoPE (all resident) ----------
          with tc.tile_pool(name="p1w", bufs=1) as p1w, \
               tc.tile_pool(name="p1x", bufs=kn["p1x"]) as p1x, \
               tc.tile_pool(name="qswp", bufs=2) as qswp, \
               tc.tile_pool(name="p1stg", bufs=1) as p1stg, \
               tc.tile_pool(name="p1ps", bufs=kn["p1ps"], space="PSUM") as p1ps, \
               tc.tile_pool(name="p1vps", bufs=kn["p1vps"], space="PSUM") as p1vps:
            # startup-critical order: xs0h per-kc2, per-mt wqk stream (h then l),
            # xs0l, xs1, wv, consts
            F8 = mybir.dt.float8e4
            DR = mybir.MatmulPerfMode.DoubleRow
            twqkh, twqkl, twvh, twvl = [], [], [], []
            wt = p1w.tile([128, 8, 2, 128], F8, tag="wqkh0", name="wqkh0")
            nc.sync.dma_start(out=wt, in_=wqkh[0])
            twqkh.append(wt)
            xs0h = p1x.tile([128, 8, 2, SLAB], F8, tag="xsh", name="xs0h")
            for kc2 in range(8):
                nc.sync.dma_start(out=xs0h[:, kc2, :, :], in_=xh[:, kc2, :, 0:SLAB])
            wt = p1w.tile([128, 8, 2, 128], F8, tag="wqkl0", name="wqkl0")
            nc.sync.dma_start(out=wt, in_=wqkl[0])
            twqkl.append(wt)
            xs0l = p1x.tile([128, 8, 2, SLAB], F8, tag="xsl", name="xs0l")
            for kc2 in range(8):
                nc.sync.dma_start(out=xs0l[:, kc2, :, :], in_=xl[:, kc2, :, 0:SLAB])
            for mt in range(1, 8):
                wt = p1w.tile([128, 8, 2, 128], F8, tag=f"wqkh{mt}")
                nc.sync.dma_start(out=wt, in_=wqkh[mt])
                twqkh.append(wt)
                wt = p1w.tile([128, 8, 2, 128], F8, tag=f"wqkl{mt}")
                nc.sync.dma_start(out=wt, in_=wqkl[mt])
                twqkl.append(wt)
            xs1h = p1x.tile([128, 8, 2, SLAB], F8, tag="xsh", name="xs1h")
            nc.sync.dma_start(out=xs1h, in_=xh[:, :, :, SLAB:2 * SLAB])
            xs1l = p1x.tile([128, 8, 2, SLAB], F8, tag="xsl", name="xs1l")
            nc.sync.dma_start(out=xs1l, in_=xl[:, :, :, SLAB:2 * SLAB])
            for kc2 in range(8):
                wt = p1w.tile([128, 2, GQ], F8, tag=f"wvh{kc2}")
                nc.sync.dma_start(out=wt, in_=wvh[kc2])
                twvh.append(wt)
                wt = p1w.tile([128, 2, GQ], F8, tag=f"wvl{kc2}")
                nc.sync.dma_start(out=wt, in_=wvl[kc2])
                twvl.append(wt)
            load_consts()
            xs_pre = {0: (xs0h, xs0l), 1: (xs1h, xs1l)}
            for ns in range(S // SLAB):
                sl = slice(ns * SLAB, (ns + 1) * SLAB)
                if ns in xs_pre:
                    xsh, xsl = xs_pre[ns]
                else:
                    xsh = p1x.tile([128, 8, 2, SLAB], F8, tag="xsh", name=f"xs{ns}h")
                    nc.sync.dma_start(out=xsh, in_=xh[:, :, :, sl])
                    xsl = p1x.tile([128, 8, 2, SLAB], F8, tag="xsl", name=f"xs{ns}l")
                    nc.sync.dma_start(out=xsl, in_=xl[:, :, :, sl])
                for mt in range(2 * HG):
                    pqk = p1ps.tile([128, SLAB], F32, tag="qkps")
                    passes = [(twqkh[mt], xsh), (twqkh[mt], xsl), (twqkl[mt], xsh)]
                    np_ = len(passes)
                    for pi, (wt, xt_) in enumerate(passes):
                        for kc2 in range(8):
                            nc.tensor.matmul(pqk[:], wt[:, kc2, :, :], xt_[:, kc2, :, :],
                                             start=(pi == 0 and kc2 == 0),
                                             stop=(pi == np_ - 1 and kc2 == 7),
                                             perf_mode=DR)
                    nc.scalar.activation(out=qt[mt][:, sl], in_=pqk[:],
                                         func=mybir.ActivationFunctionType.Identity,
                                         scale=INV_SC, bias=tbqkt[:, mt:mt + 1])
                for st in range(SLAB // 128):
                    pv = p1vps.tile([128, GQ], F32, tag="vps")
                    s0 = st * 128
                    passes = [(xsh, twvh), (xsl, twvh), (xsh, twvl)]
                    np_ = len(passes)
                    for pi, (xt_, wv_) in enumerate(passes):
                        for kc2 in range(8):
                            nc.tensor.matmul(pv[:], xt_[:, kc2, :, s0:s0 + 128],
                                             wv_[kc2][:], start=(pi == 0 and kc2 == 0),
                                             stop=(pi == np_ - 1 and kc2 == 7),
                                             perf_mode=DR)
                    nc.vector.scalar_tensor_tensor(
                        out=vres[ns * (SLAB // 128) + st], in0=pv[:],
                        scalar=tinv[:], in1=tbvb[:], op0=MULT, op1=ADD)
            # RoPE per head tile (full width): q' = qb*cos + swap(qb)*sin_rot
            # (bias already applied in the psum copy); head 0's q,k first
            for mt in (0, 4, 1, 5, 2, 6, 3, 7):
                qsw = qswp.tile([128, S], BF16, tag="qsw")
                nc.sync.dma_start(out=qsw[0:64, :], in_=qt[mt][64:128, :])
                nc.sync.dma_start(out=qsw[64:128, :], in_=qt[mt][0:64, :])
                m1 = p1stg.tile([128, S], BF16, tag="m1")
                nc.vector.tensor_tensor(out=m1[:], in0=qt[mt][:], in1=tcos[:], op=MULT)
                m2 = p1stg.tile([128, S], BF16, tag="m2")
                nc.vector.tensor_tensor(out=m2[:], in0=qsw[:], in1=tsin[:], op=MULT)
                nc.vector.tensor_tensor(out=qt[mt][:], in0=m1[:], in1=m2[:], op=ADD)

          # ---------------- Phase 2: transposed attention ----------------
          p2stack = ExitStack()
          expp = p2stack.enter_context(tc.tile_pool(name="expp", bufs=2))
          lrp = p2stack.enter_context(tc.tile_pool(name="lrp", bufs=2))
          avps = p2stack.enter_context(
              tc.tile_pool(name="avps", bufs=kn["avps"], space="PSUM"))
          rbp = p2stack.enter_context(tc.tile_pool(name="rbp", bufs=2))
          p2inner = ExitStack()
          sps = p2inner.enter_context(
              tc.tile_pool(name="sps", bufs=kn["sps"], space="PSUM"))
          smps = p2inner.enter_context(tc.tile_pool(name="smps", bufs=1, space="PSUM"))
          smt = smps.tile([128, 132], F32, tag="sm", name="smt")

          nc.sync.dma_start(out=two, in_=wo.rearrange("kc p f -> p kc f"))
          expT = [None] * HG  # per live head: list of 16 exp(S^T) tiles
          recrow = [None] * HG

          def rec_group(h, gq):
              # ship recip(ell) for q-blocks 4g..4g+3 to DRAM and back as a row
              rt = smt[0:4, 0:128]
              nc.tensor.transpose(rt, lrec_cur[h][:, 4 * gq:4 * gq + 4], ident_f[:])
              rts = lrp.tile([4, 128], F32, tag="rts")
              nc.vector.tensor_copy(out=rts[:], in_=rt)
              nc.sync.dma_start(out=lrt[h, 0, 4 * gq:4 * gq + 4, :], in_=rts[:])
              nc.sync.dma_start(out=recrow[h][:, 4 * gq:4 * gq + 4, :],
                                in_=lrt[h, :, 4 * gq:4 * gq + 4, :])

          def denom(h, b):
              # ell[q] for q-block b: sum_k exp tiles, ap-1 matmuls, then recip
              lp = smt[:, 128 + (b % 4):129 + (b % 4)]
              for j in range(b + 1):
                  nc.tensor.matmul(lp, expT[h][j][:, (b - j) * 128:(b - j + 1) * 128],
                                   tones[:], start=(j == 0), stop=(j == b))
              nc.vector.reciprocal(out=lrec_cur[h][:, b:b + 1], in_=lp)

          def sweep2_group(h, gq):
              # normalize+accumulate: ct = (sum_k V^T[k] expS^T[k]) * recip(ell)
              rbs = rbp.tile([128, 512], F32, tag="rbs")
              nc.gpsimd.partition_broadcast(
                  rbs[:], recrow[h][:, 4 * gq:4 * gq + 4, :])
              ct = avps.tile([128, 512], F32, tag="ct")
              last = 4 * gq + 3
              for j in range(last + 1):
                  if j <= 4 * gq:
                      nc.tensor.matmul(ct[:], vres[j][:, h * 128:(h + 1) * 128],
                                       expT[h][j][:, (4 * gq - j) * 128:(4 * gq - j) * 128 + 512],
                                       start=(j == 0), stop=(j == last))
                  else:
                      w = (4 * gq + 4 - j) * 128
                      nc.tensor.matmul(ct[:, 512 - w:512], vres[j][:, h * 128:(h + 1) * 128],
                                       expT[h][j][:, 0:w], start=False, stop=(j == last))
              nc.vector.tensor_tensor(out=ct_sb[(h, gq)][:], in0=ct[:], in1=rbs[:], op=MULT)

          lrec_cur = {}
          for h in range(HG):
              expT[h] = []
              lrec_cur[h] = lrp.tile([128, 16], F32, tag="lrec", name="lrec")
              recrow[h] = lrp.tile([1, 16, 128], F32, tag="recrow", name="recrow")
              for i in range(NB):
                  w = (NB - i) * 128
                  ex = expp.tile([128, w], BF16, tag=f"expT{i}", name=f"expT{i}")
                  expT[h].append(ex)
                  for c0 in range(0, w, 1024):
                      cw = min(1024, w - c0)
                      sp = sps.tile([128, 1024], F32, tag="sp")
                      for s5 in range(0, cw, 512):
                          w5 = min(512, cw - s5)
                          q0 = i * 128 + c0 + s5
                          nc.tensor.matmul(sp[:, s5:s5 + w5],
                                           qt[HG + h][:, i * 128:(i + 1) * 128],
                                           qt[h][:, q0:q0 + w5], start=True, stop=True)
                      if c0 == 0:
                          nc.gpsimd.tensor_tensor(out=sp[:, 0:128], in0=sp[:, 0:128],
                                                  in1=tmask[:], op=ADD)
                      nc.scalar.activation(out=ex[:, c0:c0 + cw], in_=sp[:, 0:cw],
                                           func=mybir.ActivationFunctionType.Exp,
                                           scale=SCALE)
                  if i >= 2:
                      denom(h, i - 2)
                      if i % 4 == 1 and i >= 5:
                          rec_group(h, (i - 5) // 4)
                  if h >= 1 and i % 4 == 3:
                      sweep2_group(h - 1, i // 4)
              denom(h, NB - 2)
              denom(h, NB - 1)
              rec_group(h, 2)
              rec_group(h, 3)

          # close S^T/denom psum pools before opening phase-3 psum
          p2inner.close()

          # ---------------- Phase 3: output projection, interleaving head-3 sweep2 ----
          with tc.tile_pool(name="p3s", bufs=4) as p3s, \
               tc.tile_pool(name="p3ps", bufs=kn["p3ps"], space="PSUM") as p3ps:
              sweep2_group(HG - 1, 0)
              for gq in range(4):
                  if gq + 1 < 4:
                      sweep2_group(HG - 1, gq + 1)
                  for mt in range(16):
                      op = p3ps.tile([128, 512], F32, tag="op")
                      for kh in range(HG):
                          nc.tensor.matmul(op[:], two[:, kh, mt * 128:(mt + 1) * 128],
                                           ct_sb[(kh, gq)][:],
                                           start=(kh == 0), stop=(kh == 3))
                      ob = p3s.tile([128, 512], BF16, tag="ob")
                      if mt % 2 == 0:
                          nc.vector.tensor_copy(out=ob[:], in_=op[:])
                      else:
                          nc.scalar.copy(out=ob[:], in_=op[:])
                      nc.sync.dma_start(out=outt[mt, :, gq * 512:(gq + 1) * 512], in_=ob[:])
          p2stack.close()
    nc.finalize()
    return nc


_NC_CACHE = {}


def _get_nc(reps=1):
    if reps not in _NC_CACHE:
        _NC_CACHE[reps] = build_nc(reps)
    return _NC_CACHE[reps]


def _rope_tables(position_ids_b):
    pos = position_ids_b.astype(np.float32)
    inv_freq = (1.0 / (ROPE_THETA ** (np.arange(0, DH, 2, dtype=np.float32) / np.float32(DH))))
    ang = pos[:, None] * inv_freq[None, :]          # [S, 64]
    emb = np.concatenate([ang, ang], axis=-1)       # [S, 128]
    cosT = np.ascontiguousarray(np.cos(emb).T)      # [128, S]
    sinT = np.sin(emb).T
    sin_rot = np.concatenate([-sinT[0:64], sinT[64:128]], axis=0)
    return cosT.astype(ml_dtypes.bfloat16), np.ascontiguousarray(sin_rot).astype(ml_dtypes.bfloat16)


def _make_in_maps(inputs):
    hidden_states = np.asarray(inputs["hidden_states"], dtype=np.float32)
    position_ids = np.asarray(inputs["position_ids"])
    Wqkv = np.asarray(inputs["Wqkv"], dtype=np.float32)
    bqkv = np.asarray(inputs["bqkv"], dtype=np.float32)
    Wo = np.asarray(inputs["Wo"], dtype=np.float32)

    mask = np.tril(np.full((128, 128), NEG, dtype=np.float32), k=-1)
    tabs = [_rope_tables(np.asarray(position_ids)[b]) for b in range(B)]

    def _hilo(M, sc):
        Ms = M * np.float32(sc)
        hi = Ms.astype(ml_dtypes.float8_e4m3)
        lo = (Ms - hi.astype(np.float32)).astype(ml_dtypes.float8_e4m3)
        return hi, lo

    def _pack_pairs(M):
        # [D, C] -> [8, 128, 2, C] with row r = kc2*256 + i*128 + p
        C = M.shape[1]
        return np.ascontiguousarray(M.reshape(8, 2, 128, C).transpose(0, 2, 1, 3))

    def _pack_x(M):
        # [D, S] -> [128, 8, 2, S] partition-major
        C = M.shape[1]
        return np.ascontiguousarray(M.reshape(8, 2, 128, C).transpose(2, 0, 1, 3))

    xts = []
    for b in range(B):
        hi, lo = _hilo(np.ascontiguousarray(hidden_states[b].T), XSC)
        xts.append((_pack_x(hi.astype(np.float32)).astype(ml_dtypes.float8_e4m3),
                    _pack_x(lo.astype(np.float32)).astype(ml_dtypes.float8_e4m3)))
    onesb = np.ones((128, 1), dtype=ml_dtypes.bfloat16)

    in_maps = []
    for c in range(NCORES):
        b, hg = divmod(c, HG)
        qcols = slice(hg * GQ, (hg + 1) * GQ)
        kcols = slice(D + hg * GQ, D + (hg + 1) * GQ)
        vcols = slice(2 * D + hg * GQ, 2 * D + (hg + 1) * GQ)
        wqk_c = np.ascontiguousarray(np.concatenate([Wqkv[:, qcols], Wqkv[:, kcols]], axis=1))
        qk_h, qk_l = _hilo(wqk_c, WSC)
        # per-mt packing: [8(mt), 128(p), 8(kc2), 2(i), 128(m)]
        def _pack_mt(M8):
            P = _pack_pairs(M8.astype(np.float32))          # [8, 128, 2, 1024]
            P = P.reshape(8, 128, 2, 8, 128)                 # [kc2, p, i, mt, m]
            return np.ascontiguousarray(P.transpose(3, 1, 0, 2, 4)).astype(ml_dtypes.float8_e4m3)
        wqkh_c = _pack_mt(qk_h)
        wqkl_c = _pack_mt(qk_l)
        wv_c = np.ascontiguousarray(Wqkv[:, vcols])
        v_h, v_l = _hilo(wv_c, WSC)
        wvh_c = _pack_pairs(v_h.astype(np.float32)).astype(ml_dtypes.float8_e4m3)
        wvl_c = _pack_pairs(v_l.astype(np.float32)).astype(ml_dtypes.float8_e4m3)
        wo_c = np.ascontiguousarray(Wo[hg * GQ:(hg + 1) * GQ, :]).astype(ml_dtypes.bfloat16).reshape(4, 128, D)
        bqk_c = np.concatenate([bqkv[qcols], bqkv[kcols]]).reshape(8, 128).T
        bv_c = bqkv[vcols].reshape(1, GQ)
        cosT, sin_rot = tabs[b]
        in_maps.append({
            "xh": xts[b][0], "xl": xts[b][1],
            "wqkh": wqkh_c, "wqkl": wqkl_c, "wvh": wvh_c, "wvl": wvl_c, "wo": wo_c,
            "bqkt": np.ascontiguousarray(bqk_c),
            "bv": np.ascontiguousarray(bv_c),
            "cost": cosT, "sinrt": sin_rot, "maskd": mask,
            "onesb": onesb,
        })
    return in_maps


def kernel(hidden_states, position_ids, Wqkv, bqkv, Wo, bo, _reps=1):
    bo = np.asarray(bo, dtype=np.float32)
    in_maps = _make_in_maps({
        "hidden_states": hidden_states, "position_ids": position_ids,
        "Wqkv": Wqkv, "bqkv": bqkv, "Wo": Wo, "bo": bo,
    })
    nc = _get_nc(_reps)
    res = run_bass_kernel_spmd(nc, in_maps, core_ids=list(range(NCORES)))

    out = np.empty((B, S, D), dtype=np.float32)
    for b in range(B):
        acc = res.results[b * HG]["outt"].reshape(D, S).astype(np.float32).copy()
        for hg in range(1, HG):
            acc += res.results[b * HG + hg]["outt"].reshape(D, S).astype(np.float32)
        out[b] = acc.T + bo[None, :]
    return out


# revision 18
# speedup vs baseline: 1.3871x; 1.0094x over previous
"""Trainium2 Bass kernel for CustomRoPEAttention (B=2, S=2048, H=16, Dh=128).

Sharding: 8 cores = 2 batches x 4 head-groups (4 heads/core). Each core:
QKV projection (bf16 matmuls) + RoPE for its heads, transposed-layout causal
attention (scores computed as S^T with keys on partitions so the attention
probabilities feed A@V directly as the moving operand -- no PE transposes),
softmax denominators via ap-size-1 matmuls + deferred normalization, and a
partial (transposed) output projection. Host sums the 4 partials per batch.

Self-contained: hardcodes shapes from the problem spec.
"""
import math
from contextlib import ExitStack

import numpy as np
import ml_dtypes

import concourse.mybir as mybir
import concourse.tile as tile
from concourse import bacc
from concourse.bass_utils import run_bass_kernel_spmd
from concourse.masks import make_identity

S = 2048            # sequence
D = 2048            # hidden
NH = 16             # total heads
DH = 128            # head dim
HG = 4              # heads per core
GQ = HG * DH        # 512: per-core q/k/v feature width
B = 2
NCORES = 8
ROPE_THETA = 10000.0
SCALE = 1.0 / math.sqrt(DH)
NEG = -1.0e9
SLAB = 512          # phase-1 sequence slab width
XSC = 16.0          # fp8 pre-scale for x (keeps hi/lo in e4m3 normal range)
WSC = 512.0         # fp8 pre-scale for Wqkv
INV_SC = 1.0 / (XSC * WSC)
F32 = mybir.dt.float32
BF16 = mybir.dt.bfloat16
F16 = mybir.dt.float16
MULT = mybir.AluOpType.mult
ADD = mybir.AluOpType.add
NB = S // 128       # 16 k/q blocks


def build_nc(reps=1, knobs=None):
    kn = {"p1ps": 3, "p1vps": 2, "p1x": 2, "sps": 3, "avps": 1, "p3ps": 4}
    if knobs:
        kn.update(knobs)
    nc = bacc.Bacc(None, target_bir_lowering=False)
    F8 = mybir.dt.float8e4
    xh = nc.dram_tensor("xh", [128, 8, 2, S], F8, kind="ExternalInput")
    xl = nc.dram_tensor("xl", [128, 8, 2, S], F8, kind="ExternalInput")
    # per-mt packed qk weights: [mt, p, kc2, i, m]
    wqkh = nc.dram_tensor("wqkh", [8, 128, 8, 2, 128], F8, kind="ExternalInput")
    wqkl = nc.dram_tensor("wqkl", [8, 128, 8, 2, 128], F8, kind="ExternalInput")
    wvh = nc.dram_tensor("wvh", [8, 128, 2, GQ], F8, kind="ExternalInput")
    wvl = nc.dram_tensor("wvl", [8, 128, 2, GQ], F8, kind="ExternalInput")
    wo = nc.dram_tensor("wo", [4, 128, D], BF16, kind="ExternalInput")
    bqkt = nc.dram_tensor("bqkt", [128, 8], F32, kind="ExternalInput")
    bv = nc.dram_tensor("bv", [1, GQ], F32, kind="ExternalInput")
    cost = nc.dram_tensor("cost", [128, S], BF16, kind="ExternalInput")    # cos^T
    sinrt = nc.dram_tensor("sinrt", [128, S], BF16, kind="ExternalInput")  # sin^T, rot sign
    maskd = nc.dram_tensor("maskd", [128, 128], F32, kind="ExternalInput")  # tril(-1) NEG
    onesb = nc.dram_tensor("onesb", [128, 1], BF16, kind="ExternalInput")
    outt = nc.dram_tensor("outt", [16, 128, S], BF16, kind="ExternalOutput")
    lrt = nc.dram_tensor("lrt", [HG, 1, 16, 128], F32)  # recip bounce: [16,128] -> [1,2048]

    with tile.TileContext(nc) as tc, ExitStack() as top:
        g = top.enter_context(tc.tile_pool(name="glob", bufs=1))
        tcos = g.tile([128, S], BF16)
        tsin = g.tile([128, S], BF16)
        tmask = g.tile([128, 128], F32)
        ident_f = g.tile([128, 128], F32)
        make_identity(nc, ident_f[:])
        tbqkt = g.tile([128, 8], F32)
        nc.sync.dma_start(out=tbqkt, in_=bqkt[:])
        tbvb = g.tile([128, GQ], F32)
        tones = g.tile([128, 1], BF16)
        tinv = g.tile([128, 1], F32)
        nc.vector.memset(tinv[:], INV_SC)

        def load_consts():
            nc.sync.dma_start(out=tcos, in_=cost[:])
            nc.sync.dma_start(out=tsin, in_=sinrt[:])
            nc.sync.dma_start(out=tmask, in_=maskd[:])
            nc.sync.dma_start(out=tbvb, in_=bv[:].to_broadcast((128, GQ)))
            nc.sync.dma_start(out=tones, in_=onesb[:])

        # Whole-kernel residents
        res = top.enter_context(tc.tile_pool(name="res", bufs=1))
        qt = []  # mt 0..3 = Q^T heads, 4..7 = K^T heads, each [128(dh), S] bf16
        for mt in range(2 * HG):
            qt.append(res.tile([128, S], BF16, tag=f"qt{mt}", name=f"qt{mt}"))
        vres = []  # 16 V k-block tiles [128(seq), GQ] bf16
        for t in range(NB):
            vres.append(res.tile([128, GQ], BF16, tag=f"v{t}", name=f"v{t}"))
        two = res.tile([128, 4, D], BF16, tag="two")
        ct_sb = {}
        for h in range(HG):
            for gq in range(4):
                ct_sb[(h, gq)] = res.tile([128, 512], BF16, tag=f"ct_{h}_{gq}", name=f"ct_{h}_{gq}")

        for _rep in range(reps):
          # ---------------- Phase 1: QKV^T projection + RoPE (all resident) ----------
          with tc.tile_pool(name="p1w", bufs=1) as p1w, \
               tc.tile_pool(name="p1x", bufs=kn["p1x"]) as p1x, \
               tc.tile_pool(name="qswp", bufs=2) as qswp, \
               tc.tile_pool(name="p1stg", bufs=1) as p1stg, \
               tc.tile_pool(name="p1ps", bufs=kn["p1ps"], space="PSUM") as p1ps, \
               tc.tile_pool(name="p1vps", bufs=kn["p1vps"], space="PSUM") as p1vps:
            # startup-critical order: xs0h per-kc2, per-mt wqk stream (h then l),
            # xs0l, xs1, wv, consts
            F8 = mybir.dt.float8e4
            DR = mybir.MatmulPerfMode.DoubleRow
            twqkh, twqkl, twvh, twvl = [], [], [], []
            wt = p1w.tile([128, 8, 2, 128], F8, tag="wqkh0", name="wqkh0")
            nc.sync.dma_start(out=wt, in_=wqkh[0])
            twqkh.append(wt)
            xs0h = p1x.tile([128, 8, 2, SLAB], F8, tag="xsh", name="xs0h")
            for kc2 in range(8):
                nc.sync.dma_start(out=xs0h[:, kc2, :, :], in_=xh[:, kc2, :, 0:SLAB])
            wt = p1w.tile([128, 8, 2, 128], F8, tag="wqkl0", name="wqkl0")
            nc.sync.dma_start(out=wt, in_=wqkl[0])
            twqkl.append(wt)
            xs0l = p1x.tile([128, 8, 2, SLAB], F8, tag="xsl", name="xs0l")
            for kc2 in range(8):
                nc.sync.dma_start(out=xs0l[:, kc2, :, :], in_=xl[:, kc2, :, 0:SLAB])
            for mt in range(1, 8):
                wt = p1w.tile([128, 8, 2, 128], F8, tag=f"wqkh{mt}")
                nc.sync.dma_start(out=wt, in_=wqkh[mt])
                twqkh.append(wt)
                wt = p1w.tile([128, 8, 2, 128], F8, tag=f"wqkl{mt}")
                nc.sync.dma_start(out=wt, in_=wqkl[mt])
                twqkl.append(wt)
            xs1h = p1x.tile([128, 8, 2, SLAB], F8, tag="xsh", name="xs1h")
            nc.sync.dma_start(out=xs1h, in_=xh[:, :, :, SLAB:2 * SLAB])
            xs1l = p1x.tile([128, 8, 2, SLAB], F8, tag="xsl", name="xs1l")
            nc.sync.dma_start(out=xs1l, in_=xl[:, :, :, SLAB:2 * SLAB])
            for kc2 in range(8):
                wt = p1w.tile([128, 2, GQ], F8, tag=f"wvh{kc2}")
                nc.sync.dma_start(out=wt, in_=wvh[kc2])
                twvh.append(wt)
                wt = p1w.tile([128, 2, GQ], F8, tag=f"wvl{kc2}")
                nc.sync.dma_start(out=wt, in_=wvl[kc2])
                twvl.append(wt)
            load_consts()
            xs_pre = {0: (xs0h, xs0l), 1: (xs1h, xs1l)}
            for ns in range(S // SLAB):
                sl = slice(ns * SLAB, (ns + 1) * SLAB)
                if ns in xs_pre:
                    xsh, xsl = xs_pre[ns]
                else:
                    xsh = p1x.tile([128, 8, 2, SLAB], F8, tag="xsh", name=f"xs{ns}h")
                    nc.sync.dma_start(out=xsh, in_=xh[:, :, :, sl])
                    xsl = p1x.tile([128, 8, 2, SLAB], F8, tag="xsl", name=f"xs{ns}l")
                    nc.sync.dma_start(out=xsl, in_=xl[:, :, :, sl])
                for mt in range(2 * HG):
                    pqk = p1ps.tile([128, SLAB], F32, tag="qkps")
                    passes = [(twqkh[mt], xsh), (twqkh[mt], xsl), (twqkl[mt], xsh)]
                    np_ = len(passes)
                    for pi, (wt, xt_) in enumerate(passes):
                        for kc2 in range(8):
                            nc.tensor.matmul(pqk[:], wt[:, kc2, :, :], xt_[:, kc2, :, :],
                                             start=(pi == 0 and kc2 == 0),
                                             stop=(pi == np_ - 1 and kc2 == 7),
                                             perf_mode=DR)
                    nc.scalar.activation(out=qt[mt][:, sl], in_=pqk[:],
                                         func=mybir.ActivationFunctionType.Identity,
                                         scale=INV_SC, bias=tbqkt[:, mt:mt + 1])
                for st in range(SLAB // 128):
                    pv = p1vps.tile([128, GQ], F32, tag="vps")
                    s0 = st * 128
                    passes = [(xsh, twvh), (xsl, twvh), (xsh, twvl)]
                    np_ = len(passes)
                    for pi, (xt_, wv_) in enumerate(passes):
                        for kc2 in range(8):
                            nc.tensor.matmul(pv[:], xt_[:, kc2, :, s0:s0 + 128],
                                             wv_[kc2][:], start=(pi == 0 and kc2 == 0),
                                             stop=(pi == np_ - 1 and kc2 == 7),
                                             perf_mode=DR)
                    nc.vector.scalar_tensor_tensor(
                        out=vres[ns * (SLAB // 128) + st], in0=pv[:],
                        scalar=tinv[:], in1=tbvb[:], op0=MULT, op1=ADD)
            # RoPE per head tile (full width): q' = qb*cos + swap(qb)*sin_rot
            # (bias already applied in the psum copy); head 0's q,k first
            for mt in (0, 4, 1, 5, 2, 6, 3, 7):
                qsw = qswp.tile([128, S], BF16, tag="qsw")
                nc.sync.dma_start(out=qsw[0:64, :], in_=qt[mt][64:128, :])
                nc.sync.dma_start(out=qsw[64:128, :], in_=qt[mt][0:64, :])
                m1 = p1stg.tile([128, S], BF16, tag="m1")
                nc.vector.tensor_tensor(out=m1[:], in0=qt[mt][:], in1=tcos[:], op=MULT)
                m2 = p1stg.tile([128, S], BF16, tag="m2")
                nc.vector.tensor_tensor(out=m2[:], in0=qsw[:], in1=tsin[:], op=MULT)
                nc.vector.tensor_tensor(out=qt[mt][:], in0=m1[:], in1=m2[:], op=ADD)

          # ---------------- Phase 2: transposed attention ----------------
          p2stack = ExitStack()
          expp = p2stack.enter_context(tc.tile_pool(name="expp", bufs=2))
          lrp = p2stack.enter_context(tc.tile_pool(name="lrp", bufs=2))
          avps = p2stack.enter_context(
              tc.tile_pool(name="avps", bufs=kn["avps"], space="PSUM"))
          rbp = p2stack.enter_context(tc.tile_pool(name="rbp", bufs=2))
          p2inner = ExitStack()
          sps = p2inner.enter_context(
              tc.tile_pool(name="sps", bufs=kn["sps"], space="PSUM"))
          smps = p2inner.enter_context(tc.tile_pool(name="smps", bufs=1, space="PSUM"))
          smt = smps.tile([128, 132], F32, tag="sm", name="smt")

          nc.sync.dma_start(out=two, in_=wo.rearrange("kc p f -> p kc f"))
          expT = [None] * HG  # per live head: list of 16 exp(S^T) tiles
          recrow = [None] * HG

          def rec_group(h, gq):
              # ship recip(ell) for q-blocks 4g..4g+3 to DRAM and back as a row
              rt = smt[0:4, 0:128]
              nc.tensor.transpose(rt, lrec_cur[h][:, 4 * gq:4 * gq + 4], ident_f[:])
              rts = lrp.tile([4, 128], F32, tag="rts")
              nc.vector.tensor_copy(out=rts[:], in_=rt)
              nc.sync.dma_start(out=lrt[h, 0, 4 * gq:4 * gq + 4, :], in_=rts[:])
              nc.sync.dma_start(out=recrow[h][:, 4 * gq:4 * gq + 4, :],
                                in_=lrt[h, :, 4 * gq:4 * gq + 4, :])

          def denom(h, b):
              # ell[q] for q-block b: sum_k exp tiles, ap-1 matmuls, then recip
              lp = smt[:, 128 + (b % 4):129 + (b % 4)]
              for j in range(b + 1):
                  nc.tensor.matmul(lp, expT[h][j][:, (b - j) * 128:(b - j + 1) * 128],
                                   tones[:], start=(j == 0), stop=(j == b))
              nc.vector.reciprocal(out=lrec_cur[h][:, b:b + 1], in_=lp)

          def sweep2_group(h, gq):
              # normalize+accumulate: ct = (sum_k V^T[k] expS^T[k]) * recip(ell)
              rbs = rbp.tile([128, 512], F32, tag="rbs")
              nc.gpsimd.partition_broadcast(
                  rbs[:], recrow[h][:, 4 * gq:4 * gq + 4, :])
              ct = avps.tile([128, 512], F32, tag="ct")
              last = 4 * gq + 3
              for j in range(last + 1):
                  if j <= 4 * gq:
                      nc.tensor.matmul(ct[:], vres[j][:, h * 128:(h + 1) * 128],
                                       expT[h][j][:, (4 * gq - j) * 128:(4 * gq - j) * 128 + 512],
                                       start=(j == 0), stop=(j == last))
                  else:
                      w = (4 * gq + 4 - j) * 128
                      nc.tensor.matmul(ct[:, 512 - w:512], vres[j][:, h * 128:(h + 1) * 128],
                                       expT[h][j][:, 0:w], start=False, stop=(j == last))
              nc.vector.tensor_tensor(out=ct_sb[(h, gq)][:], in0=ct[:], in1=rbs[:], op=MULT)

          lrec_cur = {}
          for h in range(HG):
              expT[h] = []
              lrec_cur[h] = lrp.tile([128, 16], F32, tag="lrec", name="lrec")
              recrow[h] = lrp.tile([1, 16, 128], F32, tag="recrow", name="recrow")
              for i in range(NB):
                  w = (NB - i) * 128
                  ex = expp.tile([128, w], BF16, tag=f"expT{i}", name=f"expT{i}")
                  expT[h].append(ex)
                  for c0 in range(0, w, 1024):
                      cw = min(1024, w - c0)
                      sp = sps.tile([128, 1024], F32, tag="sp")
                      for s5 in range(0, cw, 512):
                          w5 = min(512, cw - s5)
                          q0 = i * 128 + c0 + s5
                          nc.tensor.matmul(sp[:, s5:s5 + w5],
                                           qt[HG + h][:, i * 128:(i + 1) * 128],
                                           qt[h][:, q0:q0 + w5], start=True, stop=True)
                      if c0 == 0:
                          nc.gpsimd.tensor_tensor(out=sp[:, 0:128], in0=sp[:, 0:128],
                                                  in1=tmask[:], op=ADD)
                      nc.scalar.activation(out=ex[:, c0:c0 + cw], in_=sp[:, 0:cw],
                                           func=mybir.ActivationFunctionType.Exp,
                                           scale=SCALE)
                  if i >= 2:
                      denom(h, i - 2)
                      if i % 4 == 1 and i >= 5:
                          rec_group(h, (i - 5) // 4)
                  if h >= 1 and i % 4 == 3:
                      sweep2_group(h - 1, i // 4)
              denom(h, NB - 2)
              denom(h, NB - 1)
              rec_group(h, 3)

          # close S^T/denom psum pools before opening phase-3 psum
          p2inner.close()

          # ---------------- Phase 3: output projection, interleaving head-3 sweep2 ----
          with tc.tile_pool(name="p3s", bufs=4) as p3s, \
               tc.tile_pool(name="p3ps", bufs=kn["p3ps"], space="PSUM") as p3ps:
              sweep2_group(HG - 1, 0)
              for gq in range(4):
                  if gq + 1 < 4:
                      sweep2_group(HG - 1, gq + 1)
                  for mt in range(16):
                      op = p3ps.tile([128, 512], F32, tag="op")
                      for kh in range(HG):
                          nc.tensor.matmul(op[:], two[:, kh, mt * 128:(mt + 1) * 128],
                                           ct_sb[(kh, gq)][:],
                                           start=(kh == 0), stop=(kh == 3))
                      ob = p3s.tile([128, 512], BF16, tag="ob")
                      if mt % 2 == 0:
                          nc.vector.tensor_copy(out=ob[:], in_=op[:])
                      else:
                          nc.scalar.copy(out=ob[:], in_=op[:])
                      nc.sync.dma_start(out=outt[mt, :, gq * 512:(gq + 1) * 512], in_=ob[:])
          p2stack.close()
    nc.finalize()
    return nc


_NC_CACHE = {}


def _get_nc(reps=1):
    if reps not in _NC_CACHE:
        _NC_CACHE[reps] = build_nc(reps)
    return _NC_CACHE[reps]


def _rope_tables(position_ids_b):
    pos = position_ids_b.astype(np.float32)
    inv_freq = (1.0 / (ROPE_THETA ** (np.arange(0, DH, 2, dtype=np.float32) / np.float32(DH))))
    ang = pos[:, None] * inv_freq[None, :]          # [S, 64]
    emb = np.concatenate([ang, ang], axis=-1)       # [S, 128]
    cosT = np.ascontiguousarray(np.cos(emb).T)      # [128, S]
    sinT = np.sin(emb).T
    sin_rot = np.concatenate([-sinT[0:64], sinT[64:128]], axis=0)
    return cosT.astype(ml_dtypes.bfloat16), np.ascontiguousarray(sin_rot).astype(ml_dtypes.bfloat16)


def _make_in_maps(inputs):
    hidden_states = np.asarray(inputs["hidden_states"], dtype=np.float32)
    position_ids = np.asarray(inputs["position_ids"])
    Wqkv = np.asarray(inputs["Wqkv"], dtype=np.float32)
    bqkv = np.asarray(inputs["bqkv"], dtype=np.float32)
    Wo = np.asarray(inputs["Wo"], dtype=np.float32)

    mask = np.tril(np.full((128, 128), NEG, dtype=np.float32), k=-1)
    tabs = [_rope_tables(np.asarray(position_ids)[b]) for b in range(B)]

    def _hilo(M, sc):
        Ms = M * np.float32(sc)
        hi = Ms.astype(ml_dtypes.float8_e4m3)
        lo = (Ms - hi.astype(np.float32)).astype(ml_dtypes.float8_e4m3)
        return hi, lo

    def _pack_pairs(M):
        # [D, C] -> [8, 128, 2, C] with row r = kc2*256 + i*128 + p
        C = M.shape[1]
        return np.ascontiguousarray(M.reshape(8, 2, 128, C).transpose(0, 2, 1, 3))

    def _pack_x(M):
        # [D, S] -> [128, 8, 2, S] partition-major
        C = M.shape[1]
        return np.ascontiguousarray(M.reshape(8, 2, 128, C).transpose(2, 0, 1, 3))

    xts = []
    for b in range(B):
        hi, lo = _hilo(np.ascontiguousarray(hidden_states[b].T), XSC)
        xts.append((_pack_x(hi.astype(np.float32)).astype(ml_dtypes.float8_e4m3),
                    _pack_x(lo.astype(np.float32)).astype(ml_dtypes.float8_e4m3)))
    onesb = np.ones((128, 1), dtype=ml_dtypes.bfloat16)

    in_maps = []
    for c in range(NCORES):
        b, hg = divmod(c, HG)
        qcols = slice(hg * GQ, (hg + 1) * GQ)
        kcols = slice(D + hg * GQ, D + (hg + 1) * GQ)
        vcols = slice(2 * D + hg * GQ, 2 * D + (hg + 1) * GQ)
        wqk_c = np.ascontiguousarray(np.concatenate([Wqkv[:, qcols], Wqkv[:, kcols]], axis=1))
        qk_h, qk_l = _hilo(wqk_c, WSC)
        # per-mt packing: [8(mt), 128(p), 8(kc2), 2(i), 128(m)]
        def _pack_mt(M8):
            P = _pack_pairs(M8.astype(np.float32))          # [8, 128, 2, 1024]
            P = P.reshape(8, 128, 2, 8, 128)                 # [kc2, p, i, mt, m]
            return np.ascontiguousarray(P.transpose(3, 1, 0, 2, 4)).astype(ml_dtypes.float8_e4m3)
        wqkh_c = _pack_mt(qk_h)
        wqkl_c = _pack_mt(qk_l)
        wv_c = np.ascontiguousarray(Wqkv[:, vcols])
        v_h, v_l = _hilo(wv_c, WSC)
        wvh_c = _pack_pairs(v_h.astype(np.float32)).astype(ml_dtypes.float8_e4m3)
        wvl_c = _pack_pairs(v_l.astype(np.float32)).astype(ml_dtypes.float8_e4m3)
        wo_c = np.ascontiguousarray(Wo[hg * GQ:(hg + 1) * GQ, :]).astype(ml_dtypes.bfloat16).reshape(4, 128, D)
        bqk_c = np.concatenate([bqkv[qcols], bqkv[kcols]]).reshape(8, 128).T
        bv_c = bqkv[vcols].reshape(1, GQ)
        cosT, sin_rot = tabs[b]
        in_maps.append({
            "xh": xts[b][0], "xl": xts[b][1],
            "wqkh": wqkh_c, "wqkl": wqkl_c, "wvh": wvh_c, "wvl": wvl_c, "wo": wo_c,
            "bqkt": np.ascontiguousarray(bqk_c),
            "bv": np.ascontiguousarray(bv_c),
            "cost": cosT, "sinrt": sin_rot, "maskd": mask,
            "onesb": onesb,
        })
    return in_maps


def kernel(hidden_states, position_ids, Wqkv, bqkv, Wo, bo, _reps=1):
    bo = np.asarray(bo, dtype=np.float32)
    in_maps = _make_in_maps({
        "hidden_states": hidden_states, "position_ids": position_ids,
        "Wqkv": Wqkv, "bqkv": bqkv, "Wo": Wo, "bo": bo,
    })
    nc = _get_nc(_reps)
    res = run_bass_kernel_spmd(nc, in_maps, core_ids=list(range(NCORES)))

    out = np.empty((B, S, D), dtype=np.float32)
    for b in range(B):
        acc = res.results[b * HG]["outt"].reshape(D, S).astype(np.float32).copy()
        for hg in range(1, HG):
            acc += res.results[b * HG + hg]["outt"].reshape(D, S).astype(np.float32)
        out[b] = acc.T + bo[None, :]
    return out


# revision 20
# speedup vs baseline: 1.5017x; 1.0826x over previous
"""Trainium2 Bass kernel for CustomRoPEAttention (B=2, S=2048, H=16, Dh=128).

Sharding: 8 cores = 2 batches x 4 head-groups (4 heads/core). Each core:
QKV projection (bf16 matmuls) + RoPE for its heads, transposed-layout causal
attention (scores computed as S^T with keys on partitions so the attention
probabilities feed A@V directly as the moving operand -- no PE transposes),
softmax denominators via ap-size-1 matmuls + deferred normalization, and a
partial (transposed) output projection. Host sums the 4 partials per batch.

Self-contained: hardcodes shapes from the problem spec.
"""
import math
from contextlib import ExitStack

import numpy as np
import ml_dtypes

import concourse.mybir as mybir
import concourse.tile as tile
from concourse import bacc
from concourse.bass_utils import run_bass_kernel_spmd
from concourse.masks import make_identity

S = 2048            # sequence
D = 2048            # hidden
NH = 16             # total heads
DH = 128            # head dim
HG = 4              # heads per core
GQ = HG * DH        # 512: per-core q/k/v feature width
B = 2
NCORES = 8
ROPE_THETA = 10000.0
SCALE = 1.0 / math.sqrt(DH)
NEG = -1.0e9
SLAB = 512          # phase-1 sequence slab width
XSC = 16.0          # fp8 pre-scale for x (keeps hi/lo in e4m3 normal range)
WSC = 512.0         # fp8 pre-scale for Wqkv
INV_SC = 1.0 / (XSC * WSC)
F32 = mybir.dt.float32
BF16 = mybir.dt.bfloat16
F16 = mybir.dt.float16
MULT = mybir.AluOpType.mult
ADD = mybir.AluOpType.add
NB = S // 128       # 16 k/q blocks


def build_nc(reps=1, knobs=None):
    kn = {"p1ps": 3, "p1vps": 2, "p1x": 2, "sps": 3, "avps": 1, "p3ps": 4}
    if knobs:
        kn.update(knobs)
    nc = bacc.Bacc(None, target_bir_lowering=False)
    F8 = mybir.dt.float8e4
    xh = nc.dram_tensor("xh", [128, 8, 2, S], F8, kind="ExternalInput")
    xl = nc.dram_tensor("xl", [128, 8, 2, S], F8, kind="ExternalInput")
    # per-mt packed qk weights: [mt, p, kc2, i, m]
    wqkh = nc.dram_tensor("wqkh", [8, 128, 8, 2, 128], F8, kind="ExternalInput")
    wqkl = nc.dram_tensor("wqkl", [8, 128, 8, 2, 128], F8, kind="ExternalInput")
    wvh = nc.dram_tensor("wvh", [8, 128, 2, GQ], F8, kind="ExternalInput")
    wvl = nc.dram_tensor("wvl", [8, 128, 2, GQ], F8, kind="ExternalInput")
    wo = nc.dram_tensor("wo", [4, 128, D], BF16, kind="ExternalInput")
    bqkt = nc.dram_tensor("bqkt", [128, 8], F32, kind="ExternalInput")
    bv = nc.dram_tensor("bv", [1, GQ], F32, kind="ExternalInput")
    cost = nc.dram_tensor("cost", [128, S], BF16, kind="ExternalInput")    # cos^T
    sinrt = nc.dram_tensor("sinrt", [128, S], BF16, kind="ExternalInput")  # sin^T, rot sign
    maskd = nc.dram_tensor("maskd", [128, 128], BF16, kind="ExternalInput")  # triu 0/1 keep-mask
    onesb = nc.dram_tensor("onesb", [128, 1], BF16, kind="ExternalInput")
    outt = nc.dram_tensor("outt", [16, 128, S], BF16, kind="ExternalOutput")
    lrt = nc.dram_tensor("lrt", [HG, 1, 16, 128], F32)  # recip bounce: [16,128] -> [1,2048]

    with tile.TileContext(nc) as tc, ExitStack() as top:
        g = top.enter_context(tc.tile_pool(name="glob", bufs=1))
        tcos = g.tile([128, S], BF16)
        tsin = g.tile([128, S], BF16)
        tmask = g.tile([128, 128], BF16)
        ident_f = g.tile([128, 128], F32)
        make_identity(nc, ident_f[:])
        tbqkt = g.tile([128, 8], F32)
        nc.sync.dma_start(out=tbqkt, in_=bqkt[:])
        tbvb = g.tile([128, GQ], F32)
        tones = g.tile([128, 1], BF16)
        tinv = g.tile([128, 1], F32)
        nc.vector.memset(tinv[:], INV_SC)

        def load_consts():
            nc.sync.dma_start(out=tcos, in_=cost[:])
            nc.sync.dma_start(out=tsin, in_=sinrt[:])
            nc.sync.dma_start(out=tmask, in_=maskd[:])
            nc.sync.dma_start(out=tbvb, in_=bv[:].to_broadcast((128, GQ)))
            nc.sync.dma_start(out=tones, in_=onesb[:])

        # Whole-kernel residents
        res = top.enter_context(tc.tile_pool(name="res", bufs=1))
        qt = []  # mt 0..3 = Q^T heads, 4..7 = K^T heads, each [128(dh), S] bf16
        for mt in range(2 * HG):
            qt.append(res.tile([128, S], BF16, tag=f"qt{mt}", name=f"qt{mt}"))
        vres = []  # 16 V k-block tiles [128(seq), GQ] bf16
        for t in range(NB):
            vres.append(res.tile([128, GQ], BF16, tag=f"v{t}", name=f"v{t}"))
        two = res.tile([128, 4, D], BF16, tag="two")
        ct_sb = {}
        for h in range(HG):
            for gq in range(4):
                ct_sb[(h, gq)] = res.tile([128, 512], BF16, tag=f"ct_{h}_{gq}", name=f"ct_{h}_{gq}")

        for _rep in range(reps):
          # ---------------- Phase 1: QKV^T projection + RoPE (all resident) ----------
          with tc.tile_pool(name="p1w", bufs=1) as p1w, \
               tc.tile_pool(name="p1x", bufs=kn["p1x"]) as p1x, \
               tc.tile_pool(name="qswp", bufs=2) as qswp, \
               tc.tile_pool(name="p1stg", bufs=1) as p1stg, \
               tc.tile_pool(name="p1ps", bufs=kn["p1ps"], space="PSUM") as p1ps, \
               tc.tile_pool(name="p1vps", bufs=kn["p1vps"], space="PSUM") as p1vps:
            # startup-critical order: xs0h per-kc2, per-mt wqk stream (h then l),
            # xs0l, xs1, wv, consts
            F8 = mybir.dt.float8e4
            DR = mybir.MatmulPerfMode.DoubleRow
            twqkh, twqkl, twvh, twvl = [], [], [], []
            wt = p1w.tile([128, 8, 2, 128], F8, tag="wqkh0", name="wqkh0")
            nc.sync.dma_start(out=wt, in_=wqkh[0])
            twqkh.append(wt)
            xs0h = p1x.tile([128, 8, 2, SLAB], F8, tag="xsh", name="xs0h")
            for kc2 in range(8):
                nc.sync.dma_start(out=xs0h[:, kc2, :, :], in_=xh[:, kc2, :, 0:SLAB])
            wt = p1w.tile([128, 8, 2, 128], F8, tag="wqkl0", name="wqkl0")
            nc.sync.dma_start(out=wt, in_=wqkl[0])
            twqkl.append(wt)
            xs0l = p1x.tile([128, 8, 2, SLAB], F8, tag="xsl", name="xs0l")
            for kc2 in range(8):
                nc.sync.dma_start(out=xs0l[:, kc2, :, :], in_=xl[:, kc2, :, 0:SLAB])
            for mt in range(1, 8):
                wt = p1w.tile([128, 8, 2, 128], F8, tag=f"wqkh{mt}")
                nc.sync.dma_start(out=wt, in_=wqkh[mt])
                twqkh.append(wt)
                wt = p1w.tile([128, 8, 2, 128], F8, tag=f"wqkl{mt}")
                nc.sync.dma_start(out=wt, in_=wqkl[mt])
                twqkl.append(wt)
            xs1h = p1x.tile([128, 8, 2, SLAB], F8, tag="xsh", name="xs1h")
            nc.sync.dma_start(out=xs1h, in_=xh[:, :, :, SLAB:2 * SLAB])
            xs1l = p1x.tile([128, 8, 2, SLAB], F8, tag="xsl", name="xs1l")
            nc.sync.dma_start(out=xs1l, in_=xl[:, :, :, SLAB:2 * SLAB])
            for kc2 in range(8):
                wt = p1w.tile([128, 2, GQ], F8, tag=f"wvh{kc2}")
                nc.sync.dma_start(out=wt, in_=wvh[kc2])
                twvh.append(wt)
                wt = p1w.tile([128, 2, GQ], F8, tag=f"wvl{kc2}")
                nc.sync.dma_start(out=wt, in_=wvl[kc2])
                twvl.append(wt)
            load_consts()
            xs_pre = {0: (xs0h, xs0l), 1: (xs1h, xs1l)}
            for ns in range(S // SLAB):
                sl = slice(ns * SLAB, (ns + 1) * SLAB)
                if ns in xs_pre:
                    xsh, xsl = xs_pre[ns]
                else:
                    xsh = p1x.tile([128, 8, 2, SLAB], F8, tag="xsh", name=f"xs{ns}h")
                    nc.sync.dma_start(out=xsh, in_=xh[:, :, :, sl])
                    xsl = p1x.tile([128, 8, 2, SLAB], F8, tag="xsl", name=f"xs{ns}l")
                    nc.sync.dma_start(out=xsl, in_=xl[:, :, :, sl])
                for mt in range(2 * HG):
                    pqk = p1ps.tile([128, SLAB], F32, tag="qkps")
                    # q/k tolerate dropping the x_lo correction term (attention
                    # weights attenuate it); v keeps all three passes
                    passes = [(twqkh[mt], xsh), (twqkl[mt], xsh)]
                    np_ = len(passes)
                    for pi, (wt, xt_) in enumerate(passes):
                        for kc2 in range(8):
                            nc.tensor.matmul(pqk[:], wt[:, kc2, :, :], xt_[:, kc2, :, :],
                                             start=(pi == 0 and kc2 == 0),
                                             stop=(pi == np_ - 1 and kc2 == 7),
                                             perf_mode=DR)
                    nc.scalar.activation(out=qt[mt][:, sl], in_=pqk[:],
                                         func=mybir.ActivationFunctionType.Identity,
                                         scale=INV_SC, bias=tbqkt[:, mt:mt + 1])
                for st in range(SLAB // 128):
                    pv = p1vps.tile([128, GQ], F32, tag="vps")
                    s0 = st * 128
                    passes = [(xsh, twvh), (xsl, twvh), (xsh, twvl)]
                    np_ = len(passes)
                    for pi, (xt_, wv_) in enumerate(passes):
                        for kc2 in range(8):
                            nc.tensor.matmul(pv[:], xt_[:, kc2, :, s0:s0 + 128],
                                             wv_[kc2][:], start=(pi == 0 and kc2 == 0),
                                             stop=(pi == np_ - 1 and kc2 == 7),
                                             perf_mode=DR)
                    nc.vector.scalar_tensor_tensor(
                        out=vres[ns * (SLAB // 128) + st], in0=pv[:],
                        scalar=tinv[:], in1=tbvb[:], op0=MULT, op1=ADD)
            # RoPE per head tile (full width): q' = qb*cos + swap(qb)*sin_rot
            # (bias already applied in the psum copy); head 0's q,k first
            for mt in (0, 4, 1, 5, 2, 6, 3, 7):
                qsw = qswp.tile([128, S], BF16, tag="qsw")
                nc.sync.dma_start(out=qsw[0:64, :], in_=qt[mt][64:128, :])
                nc.sync.dma_start(out=qsw[64:128, :], in_=qt[mt][0:64, :])
                m1 = p1stg.tile([128, S], BF16, tag="m1")
                nc.vector.tensor_tensor(out=m1[:], in0=qt[mt][:], in1=tcos[:], op=MULT)
                m2 = p1stg.tile([128, S], BF16, tag="m2")
                nc.vector.tensor_tensor(out=m2[:], in0=qsw[:], in1=tsin[:], op=MULT)
                nc.vector.tensor_tensor(out=qt[mt][:], in0=m1[:], in1=m2[:], op=ADD)

          # ---------------- Phase 2: transposed attention ----------------
          p2stack = ExitStack()
          expp = p2stack.enter_context(tc.tile_pool(name="expp", bufs=2))
          lrp = p2stack.enter_context(tc.tile_pool(name="lrp", bufs=2))
          avps = p2stack.enter_context(
              tc.tile_pool(name="avps", bufs=kn["avps"], space="PSUM"))
          rbp = p2stack.enter_context(tc.tile_pool(name="rbp", bufs=2))
          p2inner = ExitStack()
          sps = p2inner.enter_context(
              tc.tile_pool(name="sps", bufs=kn["sps"], space="PSUM"))
          smps = p2inner.enter_context(tc.tile_pool(name="smps", bufs=1, space="PSUM"))
          smt = smps.tile([128, 132], F32, tag="sm", name="smt")

          nc.sync.dma_start(out=two, in_=wo.rearrange("kc p f -> p kc f"))
          expT = [None] * HG  # per live head: list of 16 exp(S^T) tiles
          recrow = [None] * HG

          def rec_group(h, gq):
              # ship recip(ell) for q-blocks 4g..4g+3 to DRAM and back as a row
              rt = smt[0:4, 0:128]
              nc.tensor.transpose(rt, lrec_cur[h][:, 4 * gq:4 * gq + 4], ident_f[:])
              rts = lrp.tile([4, 128], F32, tag="rts")
              nc.vector.tensor_copy(out=rts[:], in_=rt)
              nc.sync.dma_start(out=lrt[h, 0, 4 * gq:4 * gq + 4, :], in_=rts[:])
              nc.sync.dma_start(out=recrow[h][:, 4 * gq:4 * gq + 4, :],
                                in_=lrt[h, :, 4 * gq:4 * gq + 4, :])

          def denom(h, b):
              # ell[q] for q-block b: sum_k exp tiles, ap-1 matmuls, then recip
              lp = smt[:, 128 + (b % 4):129 + (b % 4)]
              for j in range(b + 1):
                  nc.tensor.matmul(lp, expT[h][j][:, (b - j) * 128:(b - j + 1) * 128],
                                   tones[:], start=(j == 0), stop=(j == b))
              nc.vector.reciprocal(out=lrec_cur[h][:, b:b + 1], in_=lp)

          def sweep2_group(h, gq):
              # normalize+accumulate: ct = (sum_k V^T[k] expS^T[k]) * recip(ell)
              rbs = rbp.tile([128, 512], F32, tag="rbs")
              nc.gpsimd.partition_broadcast(
                  rbs[:], recrow[h][:, 4 * gq:4 * gq + 4, :])
              ct = avps.tile([128, 512], F32, tag="ct")
              last = 4 * gq + 3
              for j in range(last + 1):
                  if j <= 4 * gq:
                      nc.tensor.matmul(ct[:], vres[j][:, h * 128:(h + 1) * 128],
                                       expT[h][j][:, (4 * gq - j) * 128:(4 * gq - j) * 128 + 512],
                                       start=(j == 0), stop=(j == last))
                  else:
                      w = (4 * gq + 4 - j) * 128
                      nc.tensor.matmul(ct[:, 512 - w:512], vres[j][:, h * 128:(h + 1) * 128],
                                       expT[h][j][:, 0:w], start=False, stop=(j == last))
              nc.vector.tensor_tensor(out=ct_sb[(h, gq)][:], in0=ct[:], in1=rbs[:], op=MULT)

          lrec_cur = {}
          for h in range(HG):
              expT[h] = []
              lrec_cur[h] = lrp.tile([128, 16], F32, tag="lrec", name="lrec")
              recrow[h] = lrp.tile([1, 16, 128], F32, tag="recrow", name="recrow")
              for i in range(NB):
                  w = (NB - i) * 128
                  ex = expp.tile([128, w], BF16, tag=f"expT{i}", name=f"expT{i}")
                  expT[h].append(ex)
                  for c0 in range(0, w, 1024):
                      cw = min(1024, w - c0)
                      sp = sps.tile([128, 1024], F32, tag="sp")
                      for s5 in range(0, cw, 512):
                          w5 = min(512, cw - s5)
                          q0 = i * 128 + c0 + s5
                          nc.tensor.matmul(sp[:, s5:s5 + w5],
                                           qt[HG + h][:, i * 128:(i + 1) * 128],
                                           qt[h][:, q0:q0 + w5], start=True, stop=True)
                      nc.scalar.activation(out=ex[:, c0:c0 + cw], in_=sp[:, 0:cw],
                                           func=mybir.ActivationFunctionType.Exp,
                                           scale=SCALE)
                      if c0 == 0:
                          nc.gpsimd.tensor_tensor(out=ex[:, 0:128], in0=ex[:, 0:128],
                                                  in1=tmask[:], op=MULT)
                  if i >= 2:
                      denom(h, i - 2)
                      if i % 4 == 1 and i >= 5:
                          rec_group(h, (i - 5) // 4)
                  if h >= 1 and i % 4 == 3:
                      sweep2_group(h - 1, i // 4)
              denom(h, NB - 2)
              denom(h, NB - 1)
              rec_group(h, 3)

          # close S^T/denom psum pools before opening phase-3 psum
          p2inner.close()

          # ---------------- Phase 3: output projection, interleaving head-3 sweep2 ----
          with tc.tile_pool(name="p3s", bufs=4) as p3s, \
               tc.tile_pool(name="p3ps", bufs=kn["p3ps"], space="PSUM") as p3ps:
              sweep2_group(HG - 1, 0)
              for gq in range(4):
                  if gq + 1 < 4:
                      sweep2_group(HG - 1, gq + 1)
                  for mt in range(16):
                      op = p3ps.tile([128, 512], F32, tag="op")
                      for kh in range(HG):
                          nc.tensor.matmul(op[:], two[:, kh, mt * 128:(mt + 1) * 128],
                                           ct_sb[(kh, gq)][:],
                                           start=(kh == 0), stop=(kh == 3))
                      ob = p3s.tile([128, 512], BF16, tag="ob")
                      if mt % 2 == 0:
                          nc.vector.tensor_copy(out=ob[:], in_=op[:])
                      else:
                          nc.scalar.copy(out=ob[:], in_=op[:])
                      nc.sync.dma_start(out=outt[mt, :, gq * 512:(gq + 1) * 512], in_=ob[:])
          p2stack.close()
    nc.finalize()
    return nc


_NC_CACHE = {}


def _get_nc(reps=1):
    if reps not in _NC_CACHE:
        _NC_CACHE[reps] = build_nc(reps)
    return _NC_CACHE[reps]


def _rope_tables(position_ids_b):
    pos = position_ids_b.astype(np.float32)
    inv_freq = (1.0 / (ROPE_THETA ** (np.arange(0, DH, 2, dtype=np.float32) / np.float32(DH))))
    ang = pos[:, None] * inv_freq[None, :]          # [S, 64]
    emb = np.concatenate([ang, ang], axis=-1)       # [S, 128]
    cosT = np.ascontiguousarray(np.cos(emb).T)      # [128, S]
    sinT = np.sin(emb).T
    sin_rot = np.concatenate([-sinT[0:64], sinT[64:128]], axis=0)
    return cosT.astype(ml_dtypes.bfloat16), np.ascontiguousarray(sin_rot).astype(ml_dtypes.bfloat16)


def _make_in_maps(inputs):
    hidden_states = np.asarray(inputs["hidden_states"], dtype=np.float32)
    position_ids = np.asarray(inputs["position_ids"])
    Wqkv = np.asarray(inputs["Wqkv"], dtype=np.float32)
    bqkv = np.asarray(inputs["bqkv"], dtype=np.float32)
    Wo = np.asarray(inputs["Wo"], dtype=np.float32)

    mask = np.triu(np.ones((128, 128), dtype=np.float32)).astype(ml_dtypes.bfloat16)
    tabs = [_rope_tables(np.asarray(position_ids)[b]) for b in range(B)]

    def _hilo(M, sc):
        Ms = M * np.float32(sc)
        hi = Ms.astype(ml_dtypes.float8_e4m3)
        lo = (Ms - hi.astype(np.float32)).astype(ml_dtypes.float8_e4m3)
        return hi, lo

    def _pack_pairs(M):
        # [D, C] -> [8, 128, 2, C] with row r = kc2*256 + i*128 + p
        C = M.shape[1]
        return np.ascontiguousarray(M.reshape(8, 2, 128, C).transpose(0, 2, 1, 3))

    def _pack_x(M):
        # [D, S] -> [128, 8, 2, S] partition-major
        C = M.shape[1]
        return np.ascontiguousarray(M.reshape(8, 2, 128, C).transpose(2, 0, 1, 3))

    xts = []
    for b in range(B):
        hi, lo = _hilo(np.ascontiguousarray(hidden_states[b].T), XSC)
        xts.append((_pack_x(hi.astype(np.float32)).astype(ml_dtypes.float8_e4m3),
                    _pack_x(lo.astype(np.float32)).astype(ml_dtypes.float8_e4m3)))
    onesb = np.ones((128, 1), dtype=ml_dtypes.bfloat16)

    in_maps = []
    for c in range(NCORES):
        b, hg = divmod(c, HG)
        qcols = slice(hg * GQ, (hg + 1) * GQ)
        kcols = slice(D + hg * GQ, D + (hg + 1) * GQ)
        vcols = slice(2 * D + hg * GQ, 2 * D + (hg + 1) * GQ)
        wqk_c = np.ascontiguousarray(np.concatenate([Wqkv[:, qcols], Wqkv[:, kcols]], axis=1))
        qk_h, qk_l = _hilo(wqk_c, WSC)
        # per-mt packing: [8(mt), 128(p), 8(kc2), 2(i), 128(m)]
        def _pack_mt(M8):
            P = _pack_pairs(M8.astype(np.float32))          # [8, 128, 2, 1024]
            P = P.reshape(8, 128, 2, 8, 128)                 # [kc2, p, i, mt, m]
            return np.ascontiguousarray(P.transpose(3, 1, 0, 2, 4)).astype(ml_dtypes.float8_e4m3)
        wqkh_c = _pack_mt(qk_h)
        wqkl_c = _pack_mt(qk_l)
        wv_c = np.ascontiguousarray(Wqkv[:, vcols])
        v_h, v_l = _hilo(wv_c, WSC)
        wvh_c = _pack_pairs(v_h.astype(np.float32)).astype(ml_dtypes.float8_e4m3)
        wvl_c = _pack_pairs(v_l.astype(np.float32)).astype(ml_dtypes.float8_e4m3)
        wo_c = np.ascontiguousarray(Wo[hg * GQ:(hg + 1) * GQ, :]).astype(ml_dtypes.bfloat16).reshape(4, 128, D)
        bqk_c = np.concatenate([bqkv[qcols], bqkv[kcols]]).reshape(8, 128).T
        bv_c = bqkv[vcols].reshape(1, GQ)
        cosT, sin_rot = tabs[b]
        in_maps.append({
            "xh": xts[b][0], "xl": xts[b][1],
            "wqkh": wqkh_c, "wqkl": wqkl_c, "wvh": wvh_c, "wvl": wvl_c, "wo": wo_c,
            "bqkt": np.ascontiguousarray(bqk_c),
            "bv": np.ascontiguousarray(bv_c),
            "cost": cosT, "sinrt": sin_rot, "maskd": mask,
            "onesb": onesb,
        })
    return in_maps


def kernel(hidden_states, position_ids, Wqkv, bqkv, Wo, bo, _reps=1):
    bo = np.asarray(bo, dtype=np.float32)
    in_maps = _make_in_maps({
        "hidden_states": hidden_states, "position_ids": position_ids,
        "Wqkv": Wqkv, "bqkv": bqkv, "Wo": Wo, "bo": bo,
    })
    nc = _get_nc(_reps)
    res = run_bass_kernel_spmd(nc, in_maps, core_ids=list(range(NCORES)))

    out = np.empty((B, S, D), dtype=np.float32)
    for b in range(B):
        acc = res.results[b * HG]["outt"].reshape(D, S).astype(np.float32).copy()
        for hg in range(1, HG):
            acc += res.results[b * HG + hg]["outt"].reshape(D, S).astype(np.float32)
        out[b] = acc.T + bo[None, :]
    return out
